# revision 1
# baseline (speedup 1.0000x reference)
"""AdderNet CNN (6x adder_conv + sync-BN + ReLU6) on 8 Trainium2 NeuronCores.

v3: relu-decomposition  -sum|d| = sum(d) - 2*sum(relu(d)),  d = x + w'
(w' = -w), so each k-tap needs ONE fused elementwise op (no abs/mask):

  - R+ tiles: relu(x + w') via tensor_scalar(add, max) on DVE (bf16, 4x
    mode; or fp8 out at 1x) and activation(Relu, bias) on ScalarE (fp8).
  - PE accumulates -2*R+ via one-hot matmuls with one-hot value -2.0:
    fp8 tiles in DoubleRow PAIRS (channels in PSUM rows 0..63, dst
    partition 0 per the dual-fp8 ISA restriction), bf16 tiles via 32-row
    PSUM quadrant tiling.
  - sum_k x_k (shared by ALL output channels!) is accumulated once per
    layer into row 0 of PSUM by ones-column matmuls over dense copies,
    stored to SBUF, and broadcast-added to all 128 PSUM rows by a single
    [1,128]-ones matmul per chunk.
  - sum_k w'(o,k) is a per-channel constant: BatchNorm is invariant to
    per-channel shifts, so it cancels entirely (no term needed).

Sharding: data-parallel over batch (2 images/core), sync-BN via tiny
AllReduce of per-channel (sum, sumsq) per layer.  Scratch tiles are
DENSE [128, M] (windowed strided reads of the parity-split activation
storage), M padded to 16B so DR pair strides are legal.  Channel issue
order is interleaved across producer paths by a greedy virtual-clock
schedule so PE always has ready work.
"""

import sys
import numpy as np

if "/opt/trn_rl_repo" not in sys.path:
    sys.path.insert(0, "/opt/trn_rl_repo")

import ml_dtypes

N_CORES = 8
N_LOC = 2  # images per core
EPS = 1e-5

# (Cin, Cout, k, stride, pad, Hi, Wi, Ho, Wo)
LAYER_SHAPES = [
    (512, 256, 1, 1, 0, 38, 38, 38, 38),
    (256, 512, 3, 2, 1, 38, 38, 19, 19),
    (512, 128, 1, 1, 0, 19, 19, 19, 19),
    (128, 256, 3, 2, 1, 19, 19, 10, 10),
    (256, 128, 1, 1, 0, 10, 10, 10, 10),
    (128, 256, 3, 2, 0, 10, 10, 4, 4),
]

# per-layer channel split per 128-block: (nA8, nE8, nD)
# A8 = Act->fp8 DR rows; E8 = DVE->fp8 DR rows (nA8+nE8 <= 64); D = DVE bf16
SPLITS = [
    (37, 5, 86),
    (37, 5, 86),
    (37, 5, 86),
    (37, 5, 86),
    (37, 5, 86),
    (37, 5, 86),
]

# chunk splits (PSUM <=512 f32, even sizes for fp8 DR alignment)
CHUNKS = [
    [482, 482, 482, 482, 482, 478],
    [362, 360],
    [362, 360],
    [200],
    [200],
    [32],
]

SQW = 1456  # f32 stats scratch cols (stats computed in <=2 pieces)

_NC_CACHE = {}


def _host_prep(inputs, shapes=LAYER_SHAPES, n_total=16):
    """Build per-core in_maps from the raw reference inputs."""
    bf16 = ml_dtypes.bfloat16
    x = np.asarray(inputs["x"], dtype=np.float32)
    maps_common = {}
    for li, (Cin, Cout, k, *_rest) in enumerate(shapes, start=1):
        w = np.asarray(inputs[f"w{li}"], dtype=np.float32)  # [Cout,Cin,k,k]
        CB = (Cin + 127) // 128
        if k == 1:
            wt = w[:, :, 0, 0].T.reshape(CB, 128, Cout)
        else:
            # kb = cb*9 + dy*3 + dx
            wt = (
                w.transpose(1, 2, 3, 0)  # [Cin, k, k, Cout]
                .reshape(CB, 128, k * k, Cout)
                .transpose(0, 2, 1, 3)  # [CB, k*k, 128, Cout]
                .reshape(CB * k * k, 128, Cout)
            )
        # negated: device computes relu(x + w') with w' = -w
        maps_common[f"w{li}t"] = np.ascontiguousarray(-wt, dtype=np.float32)
        OB = (Cout + 127) // 128
        g = np.asarray(inputs[f"g{li}"], dtype=np.float32).reshape(OB, 128).T
        b = np.asarray(inputs[f"b{li}"], dtype=np.float32).reshape(OB, 128).T
        maps_common[f"g{li}"] = np.ascontiguousarray(g)
        maps_common[f"b{li}"] = np.ascontiguousarray(b)

    Cin0, _, _, _, _, Hi, Wi, _, _ = shapes[0]
    CB0 = (Cin0 + 127) // 128
    in_maps = []
    n_cores = n_total // N_LOC
    for c in range(n_cores):
        xs = x[c * N_LOC : (c + 1) * N_LOC]  # [2, Cin, H, W]
        xt = (
            xs.transpose(1, 0, 2, 3)
            .reshape(CB0, 128, N_LOC * Hi * Wi)
            .astype(bf16)
        )
        m = dict(maps_common)
        m["xt"] = np.ascontiguousarray(xt)
        in_maps.append(m)
    return in_maps


def _build_nc(shapes=LAYER_SHAPES, n_total=16, n_cores=N_CORES, mock_cc=False,
              splits=SPLITS):
    import concourse.bacc as bacc
    import concourse.mybir as mybir
    from concourse import tile

    OP = mybir.AluOpType
    AF = mybir.ActivationFunctionType
    BF16 = mybir.dt.bfloat16
    F32 = mybir.dt.float32
    F8 = mybir.dt.float8e4
    DR = mybir.MatmulPerfMode.DoubleRow

    nc = bacc.Bacc(
        "TRN2",
        target_bir_lowering=False,
        debug=False,
        enable_asserts=False,
        num_devices=n_cores,
    )

    cfg = []
    for li, (Cin, Cout, k, stride, pad, Hi, Wi, Ho, Wo) in enumerate(shapes):
        CB = (Cin + 127) // 128
        OB = (Cout + 127) // 128
        KB = CB * k * k
        M = N_LOC * Ho * Wo
        Mpad = (M + 15) // 16 * 16
        cfg.append(
            dict(
                li=li, Cin=Cin, Cout=Cout, k=k, stride=stride, pad=pad,
                Hi=Hi, Wi=Wi, Ho=Ho, Wo=Wo, CB=CB, OB=OB, KB=KB, M=M,
                Mpad=Mpad, NM=n_total * Ho * Wo,
            )
        )

    # ---------------- DRAM I/O ----------------
    L1 = cfg[0]
    xt_d = nc.dram_tensor("xt", [L1["CB"], 128, N_LOC * L1["Hi"] * L1["Wi"]], BF16,
                          kind="ExternalInput")
    w_d, g_d, b_d = [], [], []
    for L in cfg:
        li = L["li"] + 1
        w_d.append(nc.dram_tensor(f"w{li}t", [L["KB"], 128, L["Cout"]], F32,
                                  kind="ExternalInput"))
        g_d.append(nc.dram_tensor(f"g{li}", [128, L["OB"]], F32, kind="ExternalInput"))
        b_d.append(nc.dram_tensor(f"b{li}", [128, L["OB"]], F32, kind="ExternalInput"))
    LL = cfg[-1]
    out_d = nc.dram_tensor("out", [N_LOC, LL["Cout"], LL["Ho"], LL["Wo"]], F32,
                           kind="ExternalOutput")

    # ---------------- persistent SBUF ----------------
    def sb(name, cols, dt):
        return nc.alloc_sbuf_tensor(name, [128, cols], dt)

    xsb = sb("xsb", L1["CB"] * N_LOC * L1["Hi"] * L1["Wi"], BF16)

    act = []
    act.append(sb("act1p", cfg[0]["OB"] * N_LOC * 2 * 2 * 20 * 20, BF16))
    act.append(sb("act2", cfg[1]["OB"] * N_LOC * 19 * 19, BF16))
    act.append(sb("act3p", N_LOC * 2 * 2 * 11 * 12, BF16))
    act.append(sb("act4", cfg[3]["OB"] * N_LOC * 10 * 10, BF16))
    act.append(sb("act5p", N_LOC * 2 * 2 * 5 * 6, BF16))
    act.append(sb("fin6", cfg[5]["OB"] * N_LOC * 4 * 4, F32))

    S_max = max(L["OB"] * L["M"] for L in cfg)
    S = sb("S", S_max, F32)

    w_sb = [sb(f"wsb{L['li']}", L["KB"] * L["Cout"], F32) for L in cfg]
    g_sb = [sb(f"gsb{L['li']}", L["OB"], F32) for L in cfg]
    b_sb = [sb(f"bsb{L['li']}", L["OB"], F32) for L in cfg]

    # one-hot windows; relu rows use value -2.0 (bakes the "-2*R" scaling
    # into PE), abs rows (A8 path) use +1.0
    oh = sb("oh", 63, BF16)
    oh8 = {}
    for par in ("o", "e"):
        for val in ("a", "r"):
            oh8[par + val] = sb(f"oh8{par}{val}", 2 * 192, F8)
    # ones for Sx: column (accumulate Sx) and row (broadcast Sx to relu
    # rows only; abs rows get 0)
    ones1 = sb("ones1", 1, BF16)
    onesr = sb("onesr", 128, BF16)
    # per-row BN sign: -1 for abs rows (S=+sum|d|), +1 for relu rows
    sgn = sb("sgn", 1, F32)
    # Sx per layer, bf16 [1, M] held on partition 0
    sxsb = sb("sxsb", max(L["Mpad"] for L in cfg), BF16)

    stats_sb, stats_g, abwork = {}, {}, {}
    sqfull = sb("sqfull", SQW, F32)
    spart = sb("spart", 8, F32)

    with tile.TileContext(nc) as tc:
        # ---------------- input loads ----------------
        for cb in range(L1["CB"]):
            W0 = N_LOC * L1["Hi"] * L1["Wi"]
            nc.sync.dma_start(xsb.ap()[:, cb * W0:(cb + 1) * W0], xt_d.ap()[cb])
        for L in cfg:
            li = L["li"]
            nc.sync.dma_start(
                w_sb[li].ap().rearrange("p (k c) -> p k c", k=L["KB"]),
                w_d[li].ap().rearrange("k p c -> p k c"),
            )
            nc.sync.dma_start(g_sb[li].ap(), g_d[li].ap())
            nc.sync.dma_start(b_sb[li].ap(), b_d[li].ap())
        nc.vector.memset(oh.ap(), 0.0)
        nc.vector.memset(oh.ap()[:, 31:32], -2.0)
        for par, hot in (("o", 63), ("e", 64)):
            for val, v in (("a", 1.0), ("r", -2.0)):
                t = oh8[par + val]
                nc.vector.memset(t.ap(), 0.0)
                nc.vector.memset(t.ap()[:, hot:hot + 1], v)
                nc.vector.memset(t.ap()[:, 192 + hot:192 + hot + 1], v)
        nc.vector.memset(ones1.ap(), 1.0)
        nA8_0 = splits[0][0]
        nc.vector.memset(onesr.ap(), 1.0)
        if nA8_0 > 0:
            nc.vector.memset(onesr.ap()[:, 0:nA8_0], 0.0)
        nc.vector.memset(sgn.ap(), 1.0)
        if nA8_0 > 0:
            nc.vector.memset(sgn.ap()[0:nA8_0, :], -1.0)
        nc.gpsimd.memset(act[0].ap(), 0.0)
        nc.gpsimd.memset(act[2].ap(), 0.0)
        nc.gpsimd.memset(act[4].ap(), 0.0)

        with (
            tc.tile_pool(name="a8", bufs=5) as a8p,
            tc.tile_pool(name="de", bufs=2) as dep,
            tc.tile_pool(name="dd", bufs=3) as ddp,
            tc.tile_pool(name="sx", bufs=1) as sxp,
            tc.tile_pool(name="ps", bufs=7, space="PSUM") as psp,
            tc.tile_pool(name="psx", bufs=1, space="PSUM") as psxp,
            tc.tile_pool(name="dram", bufs=2, space="DRAM") as dramp,
        ):
            # ============ source window AP per (layer, kb) ============
            def src_window(L, kb):
                li = L["li"]
                k = L["k"]
                M = L["M"]
                if li == 0:
                    W0 = N_LOC * L["Hi"] * L["Wi"]
                    return xsb.ap()[:, kb * W0:(kb + 1) * W0]
                if k == 1:
                    src = act[li - 1]
                    return src.ap()[:, kb * M:(kb + 1) * M]
                if li == 1:
                    src, CBv, R, C = act[0], L["CB"], 20, 20
                elif li == 3:
                    src, CBv, R, C = act[2], 1, 11, 12
                else:
                    src, CBv, R, C = act[4], 1, 5, 6
                Ho, Wo, pad = L["Ho"], L["Wo"], L["pad"]
                if CBv > 1:
                    v = src.ap().rearrange(
                        "p (cb n ip jp r c) -> p cb n ip jp r c",
                        cb=CBv, n=N_LOC, ip=2, jp=2, r=R, c=C)
                else:
                    v = src.ap().rearrange(
                        "p (n ip jp r c) -> p n ip jp r c",
                        n=N_LOC, ip=2, jp=2, r=R, c=C)

                def sel(d):
                    if pad == 1:
                        return (1, 0, 0) if d == 0 else ((0, 0, 0) if d == 1 else (1, 1, 1))
                    return (0, 0, 0) if d == 0 else ((1, 0, 0) if d == 1 else (0, 1, 1))

                cb, r9 = divmod(kb, 9)
                dy, dx = divmod(r9, 3)
                ipv, rs, _ = sel(dy)
                jpv, _, cs = sel(dx)
                if CBv > 1:
                    return v[:, cb, :, ipv, jpv, rs:rs + Ho, cs:cs + Wo]
                return v[:, :, ipv, jpv, rs:rs + Ho, cs:cs + Wo]

            # ============ conv layers ============
            for L in cfg:
                li, Cout, KB, M, Mpad, OB = (L["li"], L["Cout"], L["KB"],
                                             L["M"], L["Mpad"], L["OB"])
                chunks = CHUNKS[li]
                coff = [sum(chunks[:c]) for c in range(len(chunks))]
                nchunk = len(chunks)
                nA8, nE8, nD = splits[li]
                nF = nA8 + nE8
                stats_sb[li] = sb(f"stats{li}", 2 * OB, F32)
                stats_g[li] = sb(f"statsg{li}", 2 * OB, F32)
                abwork[li] = sb(f"abw{li}", 12 * OB, F32)

                Ho, Wo = L["Ho"], L["Wo"]

                # -------- Sx: sum_k x_k, shared across channels --------
                # chunk-sequential; psx row 0 accumulates ones-col matmuls
                # over dense bf16 copies of each tap window
                for c in range(nchunk):
                    c0, cN = coff[c], chunks[c]
                    psx = psxp.tile([128, cN], F32, tag="psx",
                                    padded_shape=[128, 512])
                    for kb in range(KB):
                        sxt = sxp.tile([128, Mpad], BF16, tag="sx")
                        nc.vector.tensor_scalar_add(
                            sxt[:, :M], src_window(L, kb), 0.0)
                        nc.tensor.matmul(
                            psx[0:1, :cN], ones1.ap(), sxt[:, c0:c0 + cN],
                            start=(kb == 0), stop=(kb == KB - 1),
                            skip_group_check=True)
                    nc.vector.tensor_scalar_add(
                        sxsb.ap()[0:1, c0:c0 + cN], psx[0:1, :cN], 0.0)

                def finish_stats(ob, li=li, L=L, M=M, OB=OB):
                    # stats (in <=2 pieces) + AllReduce launch
                    npiece = (M + SQW - 1) // SQW
                    for h in range(npiece):
                        h0 = h * SQW
                        hN = min(SQW, M - h0)
                        src = S.ap()[:, ob * M + h0:ob * M + h0 + hN]
                        nc.vector.tensor_scalar(
                            sqfull.ap()[:, :hN], src, 0.0, 0.0,
                            op0=OP.add, op1=OP.add,
                            accum_out=spart.ap()[:, h:h + 1],
                        )
                        nc.scalar.activation(
                            sqfull.ap()[:, :hN], src, AF.Square,
                            accum_out=spart.ap()[:, 4 + h:5 + h],
                        )
                    if npiece == 1:
                        nc.vector.tensor_scalar_add(
                            stats_sb[li].ap()[:, 2 * ob:2 * ob + 1],
                            spart.ap()[:, 0:1], 0.0)
                        nc.vector.tensor_scalar_add(
                            stats_sb[li].ap()[:, 2 * ob + 1:2 * ob + 2],
                            spart.ap()[:, 4:5], 0.0)
                    else:
                        nc.vector.tensor_tensor(
                            stats_sb[li].ap()[:, 2 * ob:2 * ob + 1],
                            spart.ap()[:, 0:1], spart.ap()[:, 1:2],
                            op=OP.add)
                        nc.vector.tensor_tensor(
                            stats_sb[li].ap()[:, 2 * ob + 1:2 * ob + 2],
                            spart.ap()[:, 4:5], spart.ap()[:, 5:6],
                            op=OP.add)
                    sti = dramp.tile([128, 2], F32, tag=f"sti{li}_{ob}",
                                     name=f"sti{li}_{ob}")
                    sto = dramp.tile([128, 2], F32, tag=f"sto{li}_{ob}",
                                     name=f"sto{li}_{ob}")
                    nc.sync.dma_start(sti[:, :],
                                      stats_sb[li].ap()[:, 2 * ob:2 * ob + 2])
                    if mock_cc:
                        nc.sync.dma_start(sto[:, :], sti[:, :])
                    else:
                        nc.gpsimd.collective_compute(
                            "AllReduce", OP.add,
                            replica_groups=[list(range(n_cores))],
                            ins=[sti.opt()], outs=[sto.opt()],
                        )
                    nc.sync.dma_start(stats_g[li].ap()[:, 2 * ob:2 * ob + 2],
                                      sto[:, :])

                def finish_apply(ob, li=li, L=L, M=M, OB=OB, Ho=Ho, Wo=Wo):
                    aw = abwork[li].ap()

                    def col(i):
                        return aw[:, i * OB + ob:i * OB + ob + 1]

                    (mean, ex2, m2, vpe, sq_, rc, u, s2, rinv, ga, _unused,
                     Bv) = [col(i) for i in range(12)]
                    inm = 1.0 / L["NM"]
                    nc.vector.tensor_scalar_mul(
                        mean, stats_g[li].ap()[:, 2 * ob:2 * ob + 1], inm)
                    nc.vector.tensor_scalar_mul(
                        ex2, stats_g[li].ap()[:, 2 * ob + 1:2 * ob + 2], inm)
                    nc.scalar.activation(m2, mean, AF.Square)
                    nc.vector.tensor_tensor(vpe, ex2, m2, op=OP.subtract)
                    nc.vector.tensor_scalar_add(vpe, vpe, EPS)
                    nc.scalar.activation(sq_, vpe, AF.Sqrt)
                    nc.vector.reciprocal(rc, sq_)
                    nc.vector.tensor_tensor(u, vpe, rc, op=OP.mult)
                    nc.vector.tensor_tensor(s2, sq_, u, op=OP.add)
                    nc.vector.tensor_scalar_mul(s2, s2, 0.5)
                    nc.vector.reciprocal(rinv, s2)
                    gcol = g_sb[li].ap()[:, ob:ob + 1]
                    bcol = b_sb[li].ap()[:, ob:ob + 1]
                    nc.vector.tensor_tensor(ga, gcol, rinv, op=OP.mult)
                    # abs rows hold S=+sum|d| (true value negated) -> A=-ga;
                    # relu rows hold true-value-up-to-shift -> A=+ga.
                    # y = A*S + (b - A*mean)
                    nc.vector.tensor_tensor(ga, ga, sgn.ap(), op=OP.mult)
                    nc.vector.tensor_tensor(Bv, mean, ga, op=OP.mult)
                    nc.vector.tensor_tensor(Bv, bcol, Bv, op=OP.subtract)

                    def apply_tsp(dst_ap, src_ap):
                        nc.vector.tensor_scalar(
                            dst_ap, src_ap, ga, Bv, OP.mult, OP.add)

                    if li in (0, 2, 4):
                        if li == 0:
                            R, C = 20, 20
                            dstv = act[0].ap().rearrange(
                                "p (obb n ip jp r c) -> p obb n ip jp r c",
                                obb=OB, n=N_LOC, ip=2, jp=2, r=R, c=C)
                        elif li == 2:
                            R, C = 11, 12
                            dstv = act[2].ap().rearrange(
                                "p (n ip jp r c) -> p n ip jp r c",
                                n=N_LOC, ip=2, jp=2, r=R, c=C)
                        else:
                            R, C = 5, 6
                            dstv = act[4].ap().rearrange(
                                "p (n ip jp r c) -> p n ip jp r c",
                                n=N_LOC, ip=2, jp=2, r=R, c=C)
                        Sv = S.ap()[:, :OB * M].rearrange(
                            "p (obb n i j) -> p obb n i j",
                            obb=OB, n=N_LOC, i=Ho, j=Wo)
                        pad = 1 if li in (0, 2) else 0
                        for bpar in (0, 1):
                            for dpar in (0, 1):
                                srcv = Sv[:, ob, :, bpar::2, dpar::2]
                                na, ncc = srcv.shape[2], srcv.shape[3]
                                if pad == 1:
                                    ipv, rs = (0, 0) if bpar == 0 else (1, 1)
                                    jpv, cs = (0, 0) if dpar == 0 else (1, 1)
                                else:
                                    ipv, rs = (0, 0) if bpar == 0 else (1, 0)
                                    jpv, cs = (0, 0) if dpar == 0 else (1, 0)
                                if li == 0:
                                    dst = dstv[:, ob, :, ipv, jpv,
                                               rs:rs + na, cs:cs + ncc]
                                else:
                                    dst = dstv[:, :, ipv, jpv,
                                               rs:rs + na, cs:cs + ncc]
                                apply_tsp(dst, srcv)
                        blk = act[li].ap().shape[1] // OB
                        tgt = act[li].ap()[:, ob * blk:(ob + 1) * blk]
                        nc.vector.tensor_scalar(tgt, tgt, 0.0, 6.0,
                                                OP.max, OP.min)
                    else:
                        dst_t = act[li] if li != 5 else act[5]
                        apply_tsp(dst_t.ap()[:, ob * M:(ob + 1) * M],
                                  S.ap()[:, ob * M:(ob + 1) * M])
                        tgt = dst_t.ap()[:, ob * M:(ob + 1) * M]
                        nc.vector.tensor_scalar(tgt, tgt, 0.0, 6.0,
                                                OP.max, OP.min)

                pending_apply = []
                for ob in range(OB):
                    pss = [
                        psp.tile([128, chunks[c]], F32, tag="ps",
                                 name=f"ps{li}_{c}",
                                 padded_shape=[128, 512])
                        for c in range(nchunk)
                    ]
                    if nF > 0:
                        region_of = lambda o_r: (
                            "dr" if o_r < 64 else o_r // 32)
                    else:
                        region_of = lambda o_r: o_r // 32
                    started = [set() for _ in range(nchunk)]

                    def mm_start(c, o_r, is_first_tap):
                        reg = region_of(o_r)
                        start = reg not in started[c] and is_first_tap
                        if start:
                            started[c].add(reg)
                        return start

                    # per-path work queues (A/E: DR pairs + odd single)
                    queues = {"A": [], "E": [], "D": []}
                    for o_r in range(128):
                        if o_r < nA8:
                            path = "A"
                        elif o_r < nF:
                            path = "E"
                        else:
                            path = "D"
                        if path in ("A", "E"):
                            for pi in range(KB // 2):
                                queues[path].append((o_r, "pair", 2 * pi))
                            if KB % 2:
                                queues[path].append((o_r, "single", KB - 1))
                        elif li > 0:
                            # pair D taps too (one scratch tile + one
                            # semaphore handoff per 2 taps)
                            for pi in range(KB // 2):
                                queues["D"].append((o_r, "pair", 2 * pi))
                            if KB % 2:
                                queues["D"].append((o_r, "single", KB - 1))
                        else:
                            for kb in range(KB):
                                queues["D"].append((o_r, "single", kb))

                    def wap(kb, o):
                        return w_sb[li].ap()[:, kb * Cout + o:
                                             kb * Cout + o + 1]

                    def emit_f8(path, item):
                        o_r = item[0]
                        o = ob * 128 + o_r
                        q = o_r
                        par = "o" if q % 2 else "e"
                        H = 63 if q % 2 else 64
                        t8 = oh8[par + ("a" if path == "A" else "r")]

                        def produce(dst, kb):
                            if path == "A":
                                nc.scalar.activation(
                                    dst, src_window(L, kb), AF.Abs,
                                    bias=wap(kb, o), scale=1.0)
                            else:
                                nc.vector.tensor_scalar(
                                    dst, src_window(L, kb), wap(kb, o),
                                    0.0, op0=OP.add, op1=OP.max)

                        pool = a8p if path == "A" else dep
                        tag = "a8" if path == "A" else "de"
                        if item[1] == "pair":
                            kb0 = item[2]
                            lhsT_dr = t8.ap().rearrange(
                                "p (t c) -> p t c", t=2)[:, :, H - q:H - q + 64]
                            scr = pool.tile([128, 2 * Mpad], F8, tag=tag)
                            for t in (0, 1):
                                kb = kb0 + t
                                produce(scr[:, t * Mpad:t * Mpad + M], kb)
                            sv = scr.rearrange("p (t m) -> p t m", t=2)
                            for c in range(nchunk):
                                c0, cN = coff[c], chunks[c]
                                st = mm_start(c, o_r, kb0 == 0)
                                nc.tensor.matmul(
                                    pss[c][0:64, :cN], lhsT_dr,
                                    sv[:, :, c0:c0 + cN],
                                    start=st, stop=False,
                                    perf_mode=DR,
                                    skip_group_check=True)
                        else:
                            kb = item[2]
                            lhsT_s = t8.ap()[:, H - q:H - q + 64]
                            scr = pool.tile([128, Mpad], F8, tag=tag)
                            produce(scr[:, :M], kb)
                            for c in range(nchunk):
                                c0, cN = coff[c], chunks[c]
                                st = mm_start(c, o_r, KB == 1)
                                nc.tensor.matmul(
                                    pss[c][0:64, :cN], lhsT_s,
                                    scr[:, c0:c0 + cN],
                                    start=st, stop=False,
                                    skip_group_check=True)

                    def emit_D(item):
                        o_r, kind, kb0 = item
                        o = ob * 128 + o_r
                        quad, q = divmod(o_r, 32)
                        lhsT = oh.ap()[:, 31 - q:63 - q]
                        ntap = 2 if kind == "pair" else 1
                        scr = ddp.tile([128, ntap * Mpad], BF16, tag="dd")
                        for t in range(ntap):
                            kb = kb0 + t
                            nc.vector.tensor_scalar(
                                scr[:, t * Mpad:t * Mpad + M],
                                src_window(L, kb), wap(kb, o),
                                0.0, op0=OP.add, op1=OP.max)
                        for t in range(ntap):
                            for c in range(nchunk):
                                c0, cN = coff[c], chunks[c]
                                st = mm_start(c, o_r, kb0 + t == 0)
                                nc.tensor.matmul(
                                    pss[c][32 * quad:32 * quad + 32, :cN],
                                    lhsT, scr[:, t * Mpad + c0:
                                              t * Mpad + c0 + cN],
                                    start=st, stop=False,
                                    tile_position=(0, 32 * quad),
                                    skip_group_check=True)

                    # greedy merge by estimated completion time
                    tACT = tDVE = 0.0

                    def est(path):
                        item = queues[path][0]
                        if path == "A":
                            n = 2 if item[1] == "pair" else 1
                            return tACT + n * (0.92 * M + 130)
                        if path == "E":
                            n = 2 if item[1] == "pair" else 1
                            return tDVE + n * (1.042 * M + 105)
                        n = 2 if item[1] == "pair" else 1
                        return tDVE + n * (0.26 * M + 105)

                    first = nF > 0
                    n_emitted = 0
                    while any(queues.values()):
                        # defer the previous ob's BN-apply chain into this
                        # ob's emission so the stats AllReduce round-trip
                        # never blocks the DVE queue
                        if n_emitted == 12 and pending_apply:
                            finish_apply(pending_apply.pop(0))
                        n_emitted += 1
                        if first:
                            p = "A" if queues["A"] else "E"
                            first = False
                        else:
                            p = min((p for p in queues if queues[p]),
                                    key=est)
                        item = queues[p].pop(0)
                        if p == "A":
                            tACT += ((2 if item[1] == "pair" else 1)
                                     * (0.92 * M + 130))
                            emit_f8(p, item)
                        elif p == "E":
                            tDVE += ((2 if item[1] == "pair" else 1)
                                     * (1.042 * M + 105))
                            emit_f8(p, item)
                        else:
                            tDVE += ((2 if item[1] == "pair" else 1)
                                     * (0.26 * M + 105))
                            emit_D(item)

                    # broadcast-add Sx to all 128 rows (one [1,128]-ones
                    # matmul per chunk), then evacuate to S
                    for c in range(nchunk):
                        c0, cN = coff[c], chunks[c]
                        nc.tensor.matmul(
                            pss[c][0:128, :cN], onesr.ap()[0:1, :],
                            sxsb.ap()[0:1, c0:c0 + cN],
                            start=False, stop=True,
                            skip_group_check=True)
                        nc.vector.tensor_scalar_add(
                            S.ap()[:, ob * M + c0:ob * M + c0 + cN],
                            pss[c][:, :cN], 0.0,
                        )
                    finish_stats(ob)
                    pending_apply.append(ob)
                for ob_ in pending_apply:
                    finish_apply(ob_)

            # ---------------- final output DMA ----------------
            hw = LL["Ho"] * LL["Wo"]
            finv = act[5].ap().rearrange("p (ob n hw) -> p ob n hw",
                                         ob=LL["OB"], n=N_LOC, hw=hw)
            dst = out_d.ap().rearrange("n (ob p) h w -> p ob n (h w)",
                                       ob=LL["OB"], p=128)
            for ob in range(LL["OB"]):
                nc.sync.dma_start(dst[:, ob], finv[:, ob])

    nc.compile()
    return nc


def _get_nc():
    if "nc" not in _NC_CACHE:
        _NC_CACHE["nc"] = _build_nc()
    return _NC_CACHE["nc"]


def kernel(**inputs) -> np.ndarray:
    import time as _time
    from concourse.bass_utils import run_bass_kernel_spmd

    nc = _get_nc()
    in_maps = _host_prep(inputs)
    last = None
    for attempt in range(3):
        try:
            res = run_bass_kernel_spmd(nc, in_maps, core_ids=list(range(N_CORES)))
            outs = [np.asarray(r["out"]).reshape(N_LOC, 256, 4, 4)
                    for r in res.results]
            return np.concatenate(outs, axis=0).astype(np.float32)
        except Exception as e:  # transient axon mesh desync: wait + retry
            last = e
            _time.sleep(20 * (attempt + 1))
    raise last


if __name__ == "__main__":
    nc = _build_nc()
    print("build + compile OK")



# revision 32
# speedup vs baseline: 24.6784x; 24.6784x over previous
"""AdderNet CNN (6x adder_conv + sync-BN + ReLU6) on 8 Trainium2 NeuronCores.

v4: thermometer-quantization.  |x-w| = x + w - 2*min(x,w), and
min(x,w) - t0 = integral of 1[t<x]*1[t<w] over the weight range
~= sum_l Delta * u_l(x) * v_l(w) with u_l = 1[x > t_l], v_l = 1[w > t_l]
on a midpoint grid t_l = t0 + (l+0.5)*Delta covering only the (clipped)
weight range.  This is a DENSE matmul over binary features with
contraction dim 128*L per tap -- one lhsT [128, Cout] per (tap, level)
covers ALL output channels, replacing the per-channel one-hot matmuls:

  S_o(m) = P_o(m) + g'(m) (+ per-channel const, which BN cancels; BN is
  also per-channel scale-invariant so the 2*Delta factor drops out)
  P[o,m] = sum_{kb,l} V[kb,l,:,o] . u[kb,l,:,m]   (integer counts, exact)
  g'(m)  = -G(m)/(2*Delta),  G(m) = sum_k |x_k(m) - t0|  (f32 pathway)

Quantization error for taps with x > w depends only on w -> per-channel
constant -> cancelled by BN.  Data-dependent noise only from taps with
x inside the tiny clipped weight range; grid is 0-aligned so the
post-ReLU6 point mass at x=0 is exact.  Weights are clipped to
+-3 sigma (the clip residual is again a BN-cancelled constant).

Per layer: levels L = [48,16,16,16,16,16]; binary V tiles (bf16) are
streamed from DRAM; u tiles produced on DVE (4x mode, 0.26ns/col);
G accumulated in f32 (Act abs + DVE adds, one f32 ones-matmul reduce
per chunk, f32 ones-broadcast back into PSUM).  BN stats/apply and the
tiny per-ob (sum,sumsq) AllReduce are unchanged from v3.

Sharding: data-parallel over batch (2 images/core), sync-BN via
AllReduce of per-channel (sum, sumsq) per layer.
"""

import sys
import numpy as np

if "/opt/trn_rl_repo" not in sys.path:
    sys.path.insert(0, "/opt/trn_rl_repo")

import ml_dtypes

N_CORES = 8
N_LOC = 2  # images per core
EPS = 1e-5

# (Cin, Cout, k, stride, pad, Hi, Wi, Ho, Wo)
LAYER_SHAPES = [
    (512, 256, 1, 1, 0, 38, 38, 38, 38),
    (256, 512, 3, 2, 1, 38, 38, 19, 19),
    (512, 128, 1, 1, 0, 19, 19, 19, 19),
    (128, 256, 3, 2, 1, 19, 19, 10, 10),
    (256, 128, 1, 1, 0, 10, 10, 10, 10),
    (128, 256, 3, 2, 0, 10, 10, 4, 4),
]

# thermometer levels per layer (even) and weight-clip in sigmas
LEVELS = [48, 16, 16, 16, 16, 16]
WCLIP = 3.0
# layers using fp8 DoubleRow matmuls (PE at 2x; u tiles + V in fp8)
FP8 = [True, True, False, False, False, False]
# producer assignment per (kb*L + l) % 16 for fp8 layers.  D=DVE is_gt
# {0,1}, P=Pool is_gt {0,1}, A=Act Sign {-1,+1} with V scaled by 0.5 on
# the host (the -0.5*sum(V) constant is per-channel -> BN cancels it).
PRODPAT = "DADPDADDADPDADDP"

# per-layer piece/chunk structure: list of (piece_col_offset, [chunk sizes])
# pieces split M so that the live PSUM tiles fit the 8-bank budget
# (fp8-DR doubles the tile count: DR writes land in rows 0:64 only).
# L1 splits by image half, L2 by image n.
PIECES = [
    [(0, [481, 481]), (962, [481, 481]), (1924, [482, 482])],
    [(0, [361]), (361, [361])],
    [(0, [361, 361])],
    [(0, [200])],
    [(0, [200])],
    [(0, [32])],
]

SQW = 1456  # f32 stats scratch cols (stats computed in <=2 pieces)

_NC_CACHE = {}


def _grid(w, L):
    """Midpoint grid over the clipped weight range, 0-aligned.
    Returns (t0, Delta, thresholds[L])."""
    c = float(min(np.abs(w).max() * 1.0001, WCLIP * w.std()))
    j0 = L // 2
    Delta = c / j0
    t0 = -j0 * Delta
    t = t0 + (np.arange(L, dtype=np.float64) + 0.5) * Delta
    return t0, Delta, t.astype(np.float32)


def _host_prep(inputs, shapes=LAYER_SHAPES, n_total=16):
    """Build per-core in_maps from the raw reference inputs."""
    bf16 = ml_dtypes.bfloat16
    x = np.asarray(inputs["x"], dtype=np.float32)
    maps_common = {}
    for li, (Cin, Cout, k, *_rest) in enumerate(shapes, start=1):
        w = np.asarray(inputs[f"w{li}"], dtype=np.float32)  # [Cout,Cin,k,k]
        CB = (Cin + 127) // 128
        if k == 1:
            wt = w[:, :, 0, 0].T.reshape(CB, 128, Cout)
        else:
            # kb = cb*9 + dy*3 + dx
            wt = (
                w.transpose(1, 2, 3, 0)  # [Cin, k, k, Cout]
                .reshape(CB, 128, k * k, Cout)
                .transpose(0, 2, 1, 3)  # [CB, k*k, 128, Cout]
                .reshape(CB * k * k, 128, Cout)
            )
        L = LEVELS[li - 1]
        t0, Delta, t = _grid(wt, L)
        # V[kb*L + l, c, o] = 1[w > t_l], binary (fp8 for DR layers)
        KB = wt.shape[0]
        V = (wt[:, None, :, :] > t[None, :, None, None])  # [KB, L, 128, Cout]
        vdt = ml_dtypes.float8_e4m3 if FP8[li - 1] else bf16
        Vf = V.reshape(KB * L, 128, Cout).astype(np.float32)
        if FP8[li - 1]:
            for kl in range(KB * L):
                if PRODPAT[kl % 16] == "A":
                    Vf[kl] *= 0.5
        maps_common[f"w{li}q"] = np.ascontiguousarray(Vf.astype(vdt))
        thr = np.zeros((128, 2 * L + 2), np.float32)
        thr[:, :L] = t[None, :]
        thr[:, L:2 * L] = -t[None, :]
        thr[:, 2 * L] = -t0
        thr[:, 2 * L + 1] = -1.0 / (2.0 * Delta)
        maps_common[f"thr{li}"] = thr
        OB = (Cout + 127) // 128
        g = np.asarray(inputs[f"g{li}"], dtype=np.float32).reshape(OB, 128).T
        b = np.asarray(inputs[f"b{li}"], dtype=np.float32).reshape(OB, 128).T
        maps_common[f"g{li}"] = np.ascontiguousarray(g)
        maps_common[f"b{li}"] = np.ascontiguousarray(b)

    Cin0, _, _, _, _, Hi, Wi, _, _ = shapes[0]
    CB0 = (Cin0 + 127) // 128
    in_maps = []
    n_cores = n_total // N_LOC
    for c in range(n_cores):
        xs = x[c * N_LOC : (c + 1) * N_LOC]  # [2, Cin, H, W]
        xt = (
            xs.transpose(1, 0, 2, 3)
            .reshape(CB0, 128, N_LOC * Hi * Wi)
            .astype(bf16)
        )
        m = dict(maps_common)
        m["xt"] = np.ascontiguousarray(xt)
        in_maps.append(m)
    return in_maps


def _build_nc(shapes=LAYER_SHAPES, n_total=16, n_cores=N_CORES, mock_cc=False):
    import concourse.bacc as bacc
    import concourse.mybir as mybir
    from concourse import tile

    OP = mybir.AluOpType
    AF = mybir.ActivationFunctionType
    BF16 = mybir.dt.bfloat16
    F32 = mybir.dt.float32
    F8 = mybir.dt.float8e4
    DR = mybir.MatmulPerfMode.DoubleRow

    nc = bacc.Bacc(
        "TRN2",
        target_bir_lowering=False,
        debug=False,
        enable_asserts=False,
        num_devices=n_cores,
    )

    cfg = []
    for li, (Cin, Cout, k, stride, pad, Hi, Wi, Ho, Wo) in enumerate(shapes):
        CB = (Cin + 127) // 128
        OB = (Cout + 127) // 128
        KB = CB * k * k
        M = N_LOC * Ho * Wo
        Mpad = (M + 15) // 16 * 16
        cfg.append(
            dict(
                li=li, Cin=Cin, Cout=Cout, k=k, stride=stride, pad=pad,
                Hi=Hi, Wi=Wi, Ho=Ho, Wo=Wo, CB=CB, OB=OB, KB=KB, M=M,
                Mpad=Mpad, NM=n_total * Ho * Wo, L=LEVELS[li],
            )
        )

    # ---------------- DRAM I/O ----------------
    L1 = cfg[0]
    xt_d = nc.dram_tensor("xt", [L1["CB"], 128, N_LOC * L1["Hi"] * L1["Wi"]], BF16,
                          kind="ExternalInput")
    w_d, g_d, b_d, thr_d = [], [], [], []
    for L in cfg:
        li = L["li"] + 1
        vdt = F8 if FP8[L["li"]] else BF16
        w_d.append(nc.dram_tensor(f"w{li}q", [L["KB"] * L["L"], 128, L["Cout"]],
                                  vdt, kind="ExternalInput"))
        thr_d.append(nc.dram_tensor(f"thr{li}", [128, 2 * L["L"] + 2], F32,
                                    kind="ExternalInput"))
        g_d.append(nc.dram_tensor(f"g{li}", [128, L["OB"]], F32, kind="ExternalInput"))
        b_d.append(nc.dram_tensor(f"b{li}", [128, L["OB"]], F32, kind="ExternalInput"))
    LL = cfg[-1]
    out_d = nc.dram_tensor("out", [N_LOC, LL["Cout"], LL["Ho"], LL["Wo"]], F32,
                           kind="ExternalOutput")

    # ---------------- persistent SBUF ----------------
    def sb(name, cols, dt):
        return nc.alloc_sbuf_tensor(name, [128, cols], dt)

    xsb = sb("xsb", L1["CB"] * N_LOC * L1["Hi"] * L1["Wi"], BF16)

    act = []
    act.append(sb("act1p", cfg[0]["OB"] * N_LOC * 2 * 2 * 20 * 20, BF16))
    act.append(sb("act2", cfg[1]["OB"] * N_LOC * 19 * 19, BF16))
    act.append(sb("act3p", N_LOC * 2 * 2 * 11 * 12, BF16))
    act.append(sb("act4", cfg[3]["OB"] * N_LOC * 10 * 10, BF16))
    act.append(sb("act5p", N_LOC * 2 * 2 * 5 * 6, BF16))
    act.append(sb("fin6", cfg[5]["OB"] * N_LOC * 4 * 4, F32))

    S_max = max(L["OB"] * L["M"] for L in cfg)
    S = sb("S", S_max, F32)

    thr_sb = [sb(f"thrsb{L['li']}", 2 * L["L"] + 2, F32) for L in cfg]
    g_sb = [sb(f"gsb{L['li']}", L["OB"], F32) for L in cfg]
    b_sb = [sb(f"bsb{L['li']}", L["OB"], F32) for L in cfg]

    # f32 ones for the G pathway
    ones1f = sb("ones1f", 1, F32)       # [128,1] column (reduce lhsT)
    onesrf = sb("onesrf", 128, F32)     # row 0 used as [1,128] (bcast lhsT)
    # G accumulation + g' row
    Mmax = max(L["M"] for L in cfg)
    gacc = sb("gacc", Mmax, F32)
    gtmp = sb("gtmp", Mmax, F32)
    grow = sb("grow", Mmax, F32)        # g' = -G/(2*Delta), row 0

    stats_sb, stats_g, abwork = {}, {}, {}
    sqfull = sb("sqfull", SQW, F32)
    spart = sb("spart", 8, F32)

    with tile.TileContext(nc) as tc:
        # ---------------- input loads ----------------
        for cb in range(L1["CB"]):
            W0 = N_LOC * L1["Hi"] * L1["Wi"]
            nc.sync.dma_start(xsb.ap()[:, cb * W0:(cb + 1) * W0], xt_d.ap()[cb])
        for L in cfg:
            li = L["li"]
            nc.sync.dma_start(thr_sb[li].ap(), thr_d[li].ap())
            nc.sync.dma_start(g_sb[li].ap(), g_d[li].ap())
            nc.sync.dma_start(b_sb[li].ap(), b_d[li].ap())
        nc.vector.memset(ones1f.ap(), 1.0)
        nc.vector.memset(onesrf.ap(), 1.0)
        nc.gpsimd.memset(act[0].ap(), 0.0)
        nc.gpsimd.memset(act[2].ap(), 0.0)
        nc.gpsimd.memset(act[4].ap(), 0.0)

        with (
            tc.tile_pool(name="u", bufs=4) as up,
            tc.tile_pool(name="u8", bufs=6) as u8p,
            tc.tile_pool(name="v", bufs=6) as vp,
            tc.tile_pool(name="ps", bufs=8, space="PSUM") as psp,
            tc.tile_pool(name="dram", bufs=2, space="DRAM") as dramp,
        ):
            # ============ source window AP per (layer, kb[, piece]) ========
            def src_window(L, kb, n=None):
                """Window for tap kb; n=None -> full M, else image-piece n."""
                li = L["li"]
                k = L["k"]
                M = L["M"]
                if li == 0:
                    W0 = N_LOC * L["Hi"] * L["Wi"]
                    v = xsb.ap()[:, kb * W0:(kb + 1) * W0]
                    if n is None:
                        return v
                    poff, chks = PIECES[0][n]
                    return v[:, poff:poff + sum(chks)]
                if k == 1:
                    src = act[li - 1]
                    return src.ap()[:, kb * M:(kb + 1) * M]
                if li == 1:
                    src, CBv, R, C = act[0], L["CB"], 20, 20
                elif li == 3:
                    src, CBv, R, C = act[2], 1, 11, 12
                else:
                    src, CBv, R, C = act[4], 1, 5, 6
                Ho, Wo, pad = L["Ho"], L["Wo"], L["pad"]
                if CBv > 1:
                    v = src.ap().rearrange(
                        "p (cb n ip jp r c) -> p cb n ip jp r c",
                        cb=CBv, n=N_LOC, ip=2, jp=2, r=R, c=C)
                else:
                    v = src.ap().rearrange(
                        "p (n ip jp r c) -> p n ip jp r c",
                        n=N_LOC, ip=2, jp=2, r=R, c=C)

                def sel(d):
                    if pad == 1:
                        return (1, 0, 0) if d == 0 else ((0, 0, 0) if d == 1 else (1, 1, 1))
                    return (0, 0, 0) if d == 0 else ((1, 0, 0) if d == 1 else (0, 1, 1))

                cb, r9 = divmod(kb, 9)
                dy, dx = divmod(r9, 3)
                ipv, rs, _ = sel(dy)
                jpv, _, cs = sel(dx)
                if CBv > 1:
                    w = v[:, cb, :, ipv, jpv, rs:rs + Ho, cs:cs + Wo]
                else:
                    w = v[:, :, ipv, jpv, rs:rs + Ho, cs:cs + Wo]
                if n is None:
                    return w
                return w[:, n]

            # deferred BN-applies of the previous layer: (apply_fn, ob).
            # Popped at the next layer's G-phase just before the first read
            # of the corresponding act channel-block, hiding the AllReduce
            # round-trip behind the G/production work of earlier blocks.
            pending = []

            def pop_applies(upto_ob):
                while pending and pending[0][1] <= upto_ob:
                    fa, ob_prev = pending.pop(0)
                    fa(ob_prev)

            # ============ conv layers ============
            for L in cfg:
                li, Cout, KB, M, Mpad, OB, Llv = (
                    L["li"], L["Cout"], L["KB"], L["M"], L["Mpad"], L["OB"],
                    L["L"])
                stats_sb[li] = sb(f"stats{li}", 2 * OB, F32)
                stats_g[li] = sb(f"statsg{li}", 2 * OB, F32)
                abwork[li] = sb(f"abw{li}", 12 * OB, F32)

                Ho, Wo = L["Ho"], L["Wo"]
                npieces = len(PIECES[li])

                def finish_stats(ob, li=li, L=L, M=M, OB=OB):
                    # stats (in <=2 pieces) + AllReduce launch
                    npiece = (M + SQW - 1) // SQW
                    for h in range(npiece):
                        h0 = h * SQW
                        hN = min(SQW, M - h0)
                        src = S.ap()[:, ob * M + h0:ob * M + h0 + hN]
                        nc.vector.tensor_scalar(
                            sqfull.ap()[:, :hN], src, 0.0, 0.0,
                            op0=OP.add, op1=OP.add,
                            accum_out=spart.ap()[:, h:h + 1],
                        )
                        nc.scalar.activation(
                            sqfull.ap()[:, :hN], src, AF.Square,
                            accum_out=spart.ap()[:, 4 + h:5 + h],
                        )
                    if npiece == 1:
                        nc.vector.tensor_scalar_add(
                            stats_sb[li].ap()[:, 2 * ob:2 * ob + 1],
                            spart.ap()[:, 0:1], 0.0)
                        nc.vector.tensor_scalar_add(
                            stats_sb[li].ap()[:, 2 * ob + 1:2 * ob + 2],
                            spart.ap()[:, 4:5], 0.0)
                    else:
                        nc.vector.tensor_tensor(
                            stats_sb[li].ap()[:, 2 * ob:2 * ob + 1],
                            spart.ap()[:, 0:1], spart.ap()[:, 1:2],
                            op=OP.add)
                        nc.vector.tensor_tensor(
                            stats_sb[li].ap()[:, 2 * ob + 1:2 * ob + 2],
                            spart.ap()[:, 4:5], spart.ap()[:, 5:6],
                            op=OP.add)
                    # stats DMAs + collective all ride the gpsimd (Pool)
                    # queue so the AllReduce wait never blocks the V-tile
                    # prefetch stream on the SP queue.
                    sti = dramp.tile([128, 2], F32, tag=f"sti{li}_{ob}",
                                     name=f"sti{li}_{ob}")
                    sto = dramp.tile([128, 2], F32, tag=f"sto{li}_{ob}",
                                     name=f"sto{li}_{ob}")
                    nc.gpsimd.dma_start(sti[:, :],
                                        stats_sb[li].ap()[:, 2 * ob:2 * ob + 2])
                    if mock_cc:
                        nc.gpsimd.dma_start(sto[:, :], sti[:, :])
                    else:
                        nc.gpsimd.collective_compute(
                            "AllReduce", OP.add,
                            replica_groups=[list(range(n_cores))],
                            ins=[sti.opt()], outs=[sto.opt()],
                        )
                    nc.gpsimd.dma_start(stats_g[li].ap()[:, 2 * ob:2 * ob + 2],
                                        sto[:, :])

                def finish_apply(ob, li=li, L=L, M=M, OB=OB, Ho=Ho, Wo=Wo):
                    aw = abwork[li].ap()

                    def col(i):
                        return aw[:, i * OB + ob:i * OB + ob + 1]

                    (mean, ex2, m2, vpe, sq_, rc, u, s2, rinv, ga, _unused,
                     Bv) = [col(i) for i in range(12)]
                    inm = 1.0 / L["NM"]
                    nc.vector.tensor_scalar_mul(
                        mean, stats_g[li].ap()[:, 2 * ob:2 * ob + 1], inm)
                    nc.vector.tensor_scalar_mul(
                        ex2, stats_g[li].ap()[:, 2 * ob + 1:2 * ob + 2], inm)
                    nc.scalar.activation(m2, mean, AF.Square)
                    nc.vector.tensor_tensor(vpe, ex2, m2, op=OP.subtract)
                    nc.vector.tensor_scalar_add(vpe, vpe, EPS)
                    nc.scalar.activation(sq_, vpe, AF.Sqrt)
                    nc.vector.reciprocal(rc, sq_)
                    nc.vector.tensor_tensor(u, vpe, rc, op=OP.mult)
                    nc.vector.tensor_tensor(s2, sq_, u, op=OP.add)
                    nc.vector.tensor_scalar_mul(s2, s2, 0.5)
                    nc.vector.reciprocal(rinv, s2)
                    gcol = g_sb[li].ap()[:, ob:ob + 1]
                    bcol = b_sb[li].ap()[:, ob:ob + 1]
                    nc.vector.tensor_tensor(ga, gcol, rinv, op=OP.mult)
                    # y = A*S + (b - A*mean)
                    nc.vector.tensor_tensor(Bv, mean, ga, op=OP.mult)
                    nc.vector.tensor_tensor(Bv, bcol, Bv, op=OP.subtract)

                    def apply_tsp(dst_ap, src_ap):
                        nc.vector.tensor_scalar(
                            dst_ap, src_ap, ga, Bv, OP.mult, OP.add)

                    if li in (0, 2, 4):
                        if li == 0:
                            R, C = 20, 20
                            dstv = act[0].ap().rearrange(
                                "p (obb n ip jp r c) -> p obb n ip jp r c",
                                obb=OB, n=N_LOC, ip=2, jp=2, r=R, c=C)
                        elif li == 2:
                            R, C = 11, 12
                            dstv = act[2].ap().rearrange(
                                "p (n ip jp r c) -> p n ip jp r c",
                                n=N_LOC, ip=2, jp=2, r=R, c=C)
                        else:
                            R, C = 5, 6
                            dstv = act[4].ap().rearrange(
                                "p (n ip jp r c) -> p n ip jp r c",
                                n=N_LOC, ip=2, jp=2, r=R, c=C)
                        Sv = S.ap()[:, :OB * M].rearrange(
                            "p (obb n i j) -> p obb n i j",
                            obb=OB, n=N_LOC, i=Ho, j=Wo)
                        pad = 1 if li in (0, 2) else 0
                        for bpar in (0, 1):
                            for dpar in (0, 1):
                                srcv = Sv[:, ob, :, bpar::2, dpar::2]
                                na, ncc = srcv.shape[2], srcv.shape[3]
                                if pad == 1:
                                    ipv, rs = (0, 0) if bpar == 0 else (1, 1)
                                    jpv, cs = (0, 0) if dpar == 0 else (1, 1)
                                else:
                                    ipv, rs = (0, 0) if bpar == 0 else (1, 0)
                                    jpv, cs = (0, 0) if dpar == 0 else (1, 0)
                                if li == 0:
                                    dst = dstv[:, ob, :, ipv, jpv,
                                               rs:rs + na, cs:cs + ncc]
                                else:
                                    dst = dstv[:, :, ipv, jpv,
                                               rs:rs + na, cs:cs + ncc]
                                apply_tsp(dst, srcv)
                        blk = act[li].ap().shape[1] // OB
                        tgt = act[li].ap()[:, ob * blk:(ob + 1) * blk]
                        nc.vector.tensor_scalar(tgt, tgt, 0.0, 6.0,
                                                OP.max, OP.min)
                    else:
                        dst_t = act[li] if li != 5 else act[5]
                        apply_tsp(dst_t.ap()[:, ob * M:(ob + 1) * M],
                                  S.ap()[:, ob * M:(ob + 1) * M])
                        tgt = dst_t.ap()[:, ob * M:(ob + 1) * M]
                        nc.vector.tensor_scalar(tgt, tgt, 0.0, 6.0,
                                                OP.max, OP.min)

                # thresholds
                tcol = lambda l: thr_sb[li].ap()[:, l:l + 1]
                ntcol = lambda l: thr_sb[li].ap()[:, Llv + l:Llv + l + 1]
                t0col = thr_sb[li].ap()[:, 2 * Llv:2 * Llv + 1]
                dcol = thr_sb[li].ap()[0:1, 2 * Llv + 1:2 * Llv + 2]

                kk = L["k"] * L["k"]

                for pi, (poff, chunks) in enumerate(PIECES[li]):
                    n_arg = pi if npieces > 1 else None
                    Mp = sum(chunks)
                    Mp_pad = (Mp + 15) // 16 * 16
                    coff = [sum(chunks[:c]) for c in range(len(chunks))]
                    nchunk = len(chunks)

                    # ---- PSUM tiles: G slots reserved first, then conv ----
                    fp8 = FP8[li]
                    gps = [psp.tile([128, chunks[c]], F32, tag="ps",
                                    name=f"gps{li}_{pi}_{c}",
                                    padded_shape=[128, 512])
                           for c in range(nchunk)]
                    pss = {}
                    for ob in range(OB):
                        for c in range(nchunk):
                            pss[(ob, c)] = psp.tile(
                                [128, chunks[c]], F32, tag="ps",
                                name=f"ps{li}_{pi}_{ob}_{c}",
                                padded_shape=[128, 512])

                    GV = 8  # levels per V DMA (HWDGE fixed cost is per instr)
                    prod_ctr = 0
                    for kb in range(KB):
                        # previous layer's BN-apply for channel-block cb must
                        # be issued before any read of that act block
                        pop_applies(kb // kk)
                        win = src_window(L, kb, n_arg)
                        # G accumulation interleaved with production so the
                        # serial abs->add chain never head-blocks the queues
                        if kb == 0:
                            nc.scalar.activation(
                                gacc.ap()[:, :Mp], win, AF.Abs,
                                bias=t0col, scale=1.0)
                        else:
                            nc.scalar.activation(
                                gtmp.ap()[:, :Mp], win, AF.Abs,
                                bias=t0col, scale=1.0)
                            nc.vector.tensor_tensor(
                                gacc.ap()[:, :Mp], gacc.ap()[:, :Mp],
                                gtmp.ap()[:, :Mp], op=OP.add)
                        for g0 in range(0, Llv, GV):
                            gN = min(GV, Llv - g0)
                            vdt = F8 if fp8 else BF16
                            vt = vp.tile([128, gN * Cout], vdt, tag="v")
                            nc.sync.dma_start(
                                vt.rearrange("p (g c) -> p g c", g=gN),
                                w_d[li].ap()[kb * Llv + g0:
                                             kb * Llv + g0 + gN].rearrange(
                                    "g p c -> p g c"))
                            vv = vt.rearrange("p (g c) -> p g c", g=gN)
                            if fp8:
                                # level pairs -> DoubleRow matmuls (128 rows)
                                for i2 in range(0, gN, 2):
                                    scr = u8p.tile([128, 2 * Mp_pad], F8,
                                                   tag="u8")
                                    for t in (0, 1):
                                        l = g0 + i2 + t
                                        dst = scr[:, t * Mp_pad:
                                                  t * Mp_pad + Mp]
                                        p = PRODPAT[(kb * Llv + l) % 16]
                                        if p == "A":
                                            nc.scalar.activation(
                                                dst, win, AF.Sign,
                                                bias=ntcol(l), scale=1.0)
                                        else:
                                            eng = (nc.gpsimd if p == "P"
                                                   else nc.vector)
                                            eng.tensor_scalar(
                                                dst, win, tcol(l), None,
                                                op0=OP.is_gt)
                                    sv = scr.rearrange("p (t m) -> p t m",
                                                       t=2)
                                    first = (kb == 0 and g0 + i2 == 0)
                                    for ob in range(OB):
                                        lhsT = vv[:, i2:i2 + 2,
                                                  ob * 128:(ob + 1) * 128]
                                        for c in range(nchunk):
                                            c0, cN = coff[c], chunks[c]
                                            nc.tensor.matmul(
                                                pss[(ob, c)][:, :cN],
                                                lhsT,
                                                sv[:, :, c0:c0 + cN],
                                                start=first, stop=False,
                                                perf_mode=DR,
                                                skip_group_check=True)
                            else:
                                for l in range(g0, g0 + gN):
                                    vo = (l - g0) * Cout
                                    ut = up.tile([128, Mp_pad], BF16,
                                                 tag="u")
                                    nc.vector.tensor_scalar(
                                        ut[:, :Mp], win, tcol(l), None,
                                        op0=OP.is_gt)
                                    first = (kb == 0 and l == 0)
                                    for ob in range(OB):
                                        for c in range(nchunk):
                                            c0, cN = coff[c], chunks[c]
                                            nc.tensor.matmul(
                                                pss[(ob, c)][:, :cN],
                                                vt[:, vo + ob * 128:
                                                   vo + (ob + 1) * 128],
                                                ut[:, c0:c0 + cN],
                                                start=first, stop=False,
                                                skip_group_check=True)

                    # ---- G reduce to row + scale to g' (per chunk) ----
                    for c in range(nchunk):
                        c0, cN = coff[c], chunks[c]
                        nc.tensor.matmul(
                            gps[c][0:1, :cN], ones1f.ap(),
                            gacc.ap()[:, c0:c0 + cN],
                            start=True, stop=True, skip_group_check=True)
                        nc.vector.tensor_scalar_mul(
                            grow.ap()[0:1, poff + c0:poff + c0 + cN],
                            gps[c][0:1, :cN], dcol)

                    # ---- broadcast g' into all psum tiles, evacuate ----
                    for ob in range(OB):
                        for c in range(nchunk):
                            c0, cN = coff[c], chunks[c]
                            nc.tensor.matmul(
                                pss[(ob, c)][:, :cN], onesrf.ap()[0:1, :],
                                grow.ap()[0:1, poff + c0:poff + c0 + cN],
                                start=False, stop=True,
                                skip_group_check=True)
                            # evacuate on Act (Identity) to keep DVE free
                            nc.scalar.activation(
                                S.ap()[:, ob * M + poff + c0:
                                       ob * M + poff + c0 + cN],
                                pss[(ob, c)][:, :cN], AF.Identity)
                        if pi == npieces - 1:
                            finish_stats(ob)
                            pending.append((finish_apply, ob))

            for fa, ob_prev in pending:
                fa(ob_prev)
            pending.clear()

            # ---------------- final output DMA ----------------
            hw = LL["Ho"] * LL["Wo"]
            finv = act[5].ap().rearrange("p (ob n hw) -> p ob n hw",
                                         ob=LL["OB"], n=N_LOC, hw=hw)
            dst = out_d.ap().rearrange("n (ob p) h w -> p ob n (h w)",
                                       ob=LL["OB"], p=128)
            for ob in range(LL["OB"]):
                nc.sync.dma_start(dst[:, ob], finv[:, ob])

    nc.compile()
    return nc


def _get_nc():
    if "nc" not in _NC_CACHE:
        _NC_CACHE["nc"] = _build_nc()
    return _NC_CACHE["nc"]


def kernel(**inputs) -> np.ndarray:
    import time as _time
    from concourse.bass_utils import run_bass_kernel_spmd

    nc = _get_nc()
    in_maps = _host_prep(inputs)
    last = None
    for attempt in range(3):
        try:
            res = run_bass_kernel_spmd(nc, in_maps, core_ids=list(range(N_CORES)))
            outs = [np.asarray(r["out"]).reshape(N_LOC, 256, 4, 4)
                    for r in res.results]
            return np.concatenate(outs, axis=0).astype(np.float32)
        except Exception as e:  # transient axon mesh desync: wait + retry
            last = e
            _time.sleep(20 * (attempt + 1))
    raise last


if __name__ == "__main__":
    nc = _build_nc()
    print("build + compile OK")


# revision 40
# speedup vs baseline: 47.2328x; 1.9139x over previous
"""AdderNet CNN (6x adder_conv + sync-BN + ReLU6) on 8 Trainium2 NeuronCores.

v4: thermometer-quantization.  |x-w| = x + w - 2*min(x,w), and
min(x,w) - t0 = integral of 1[t<x]*1[t<w] over the weight range
~= sum_l Delta * u_l(x) * v_l(w) with u_l = 1[x > t_l], v_l = 1[w > t_l]
on a midpoint grid t_l = t0 + (l+0.5)*Delta covering only the (clipped)
weight range.  This is a DENSE matmul over binary features with
contraction dim 128*L per tap -- one lhsT [128, Cout] per (tap, level)
covers ALL output channels, replacing the per-channel one-hot matmuls:

  S_o(m) = P_o(m) + g'(m) (+ per-channel const, which BN cancels; BN is
  also per-channel scale-invariant so the 2*Delta factor drops out)
  P[o,m] = sum_{kb,l} V[kb,l,:,o] . u[kb,l,:,m]   (integer counts, exact)
  g'(m)  = -G(m)/(2*Delta),  G(m) = sum_k |x_k(m) - t0|  (f32 pathway)

Quantization error for taps with x > w depends only on w -> per-channel
constant -> cancelled by BN.  Data-dependent noise only from taps with
x inside the tiny clipped weight range; grid is 0-aligned so the
post-ReLU6 point mass at x=0 is exact.  Weights are clipped to
+-3 sigma (the clip residual is again a BN-cancelled constant).

Per layer: levels L = [48,16,16,16,16,16]; binary V tiles (bf16) are
streamed from DRAM; u tiles produced on DVE (4x mode, 0.26ns/col);
G accumulated in f32 (Act abs + DVE adds, one f32 ones-matmul reduce
per chunk, f32 ones-broadcast back into PSUM).  BN stats/apply and the
tiny per-ob (sum,sumsq) AllReduce are unchanged from v3.

Sharding: data-parallel over batch (2 images/core), sync-BN via
AllReduce of per-channel (sum, sumsq) per layer.
"""

import sys
import numpy as np

if "/opt/trn_rl_repo" not in sys.path:
    sys.path.insert(0, "/opt/trn_rl_repo")

import ml_dtypes

N_CORES = 8
N_LOC = 2  # images per core
EPS = 1e-5

# (Cin, Cout, k, stride, pad, Hi, Wi, Ho, Wo)
LAYER_SHAPES = [
    (512, 256, 1, 1, 0, 38, 38, 38, 38),
    (256, 512, 3, 2, 1, 38, 38, 19, 19),
    (512, 128, 1, 1, 0, 19, 19, 19, 19),
    (128, 256, 3, 2, 1, 19, 19, 10, 10),
    (256, 128, 1, 1, 0, 10, 10, 10, 10),
    (128, 256, 3, 2, 0, 10, 10, 4, 4),
]

# thermometer levels per layer (even) and weight-clip in sigmas
LEVELS = [36, 12, 12, 12, 12, 12]
WCLIP = 3.0
# layers using fp8 DoubleRow matmuls (PE at 2x; u tiles + V in fp8)
FP8 = [False, False, False, False, False, False]
# producer assignment per (kb*L + l) % 16 for fp8 layers.  D=DVE is_gt
# {0,1}, P=Pool is_gt {0,1}, A=Act Sign {-1,+1} with V scaled by 0.5 on
# the host (the -0.5*sum(V) constant is per-channel -> BN cancels it).
PRODPAT = "DADPDADDADPDADDP"

# per-layer piece/chunk structure: list of (piece_col_offset, [chunk sizes])
# pieces split M so that the live PSUM tiles fit the 8-bank budget
# (fp8-DR doubles the tile count: DR writes land in rows 0:64 only).
# L1 splits by image half, L2 by image n.
PIECES = [
    [(0, [481, 481]), (962, [481, 481]), (1924, [482, 482])],
    [(0, [361]), (361, [361])],
    [(0, [361, 361])],
    [(0, [200])],
    [(0, [200])],
    [(0, [32])],
]

SQW = 1456  # f32 stats scratch cols (stats computed in <=2 pieces)

_NC_CACHE = {}


def _grid(w, L):
    """Midpoint grid over the clipped weight range, 0-aligned.
    Returns (t0, Delta, thresholds[L])."""
    c = float(min(np.abs(w).max() * 1.0001, WCLIP * w.std()))
    j0 = L // 2
    Delta = c / j0
    t0 = -j0 * Delta
    t = t0 + (np.arange(L, dtype=np.float64) + 0.5) * Delta
    return t0, Delta, t.astype(np.float32)


def _host_prep(inputs, shapes=LAYER_SHAPES, n_total=16):
    """Build per-core in_maps from the raw reference inputs."""
    bf16 = ml_dtypes.bfloat16
    x = np.asarray(inputs["x"], dtype=np.float32)
    maps_common = {}
    for li, (Cin, Cout, k, *_rest) in enumerate(shapes, start=1):
        w = np.asarray(inputs[f"w{li}"], dtype=np.float32)  # [Cout,Cin,k,k]
        CB = (Cin + 127) // 128
        if k == 1:
            wt = w[:, :, 0, 0].T.reshape(CB, 128, Cout)
        else:
            # kb = cb*9 + dy*3 + dx
            wt = (
                w.transpose(1, 2, 3, 0)  # [Cin, k, k, Cout]
                .reshape(CB, 128, k * k, Cout)
                .transpose(0, 2, 1, 3)  # [CB, k*k, 128, Cout]
                .reshape(CB * k * k, 128, Cout)
            )
        L = LEVELS[li - 1]
        t0, Delta, t = _grid(wt, L)
        # V[kb*L + l, c, o] = 1[w > t_l], binary (fp8 for DR layers)
        KB = wt.shape[0]
        V = (wt[:, None, :, :] > t[None, :, None, None])  # [KB, L, 128, Cout]
        vdt = ml_dtypes.float8_e4m3 if FP8[li - 1] else bf16
        Vf = V.reshape(KB * L, 128, Cout).astype(np.float32)
        if FP8[li - 1]:
            for kl in range(KB * L):
                if PRODPAT[kl % 16] == "A":
                    Vf[kl] *= 0.5
        maps_common[f"w{li}q"] = np.ascontiguousarray(Vf.astype(vdt))
        thr = np.zeros((128, 2 * L + 2), np.float32)
        thr[:, :L] = t[None, :]
        thr[:, L:2 * L] = -t[None, :]
        thr[:, 2 * L] = -t0
        thr[:, 2 * L + 1] = -1.0 / (2.0 * Delta)
        maps_common[f"thr{li}"] = thr
        OB = (Cout + 127) // 128
        g = np.asarray(inputs[f"g{li}"], dtype=np.float32).reshape(OB, 128).T
        b = np.asarray(inputs[f"b{li}"], dtype=np.float32).reshape(OB, 128).T
        maps_common[f"g{li}"] = np.ascontiguousarray(g)
        maps_common[f"b{li}"] = np.ascontiguousarray(b)

    Cin0, _, _, _, _, Hi, Wi, _, _ = shapes[0]
    CB0 = (Cin0 + 127) // 128
    in_maps = []
    n_cores = n_total // N_LOC
    for c in range(n_cores):
        xs = x[c * N_LOC : (c + 1) * N_LOC]  # [2, Cin, H, W]
        xt = (
            xs.transpose(1, 0, 2, 3)
            .reshape(CB0, 128, N_LOC * Hi * Wi)
            .astype(bf16)
        )
        m = dict(maps_common)
        m["xt"] = np.ascontiguousarray(xt)
        in_maps.append(m)
    return in_maps


def _build_nc(shapes=LAYER_SHAPES, n_total=16, n_cores=N_CORES, mock_cc=False):
    import concourse.bacc as bacc
    import concourse.mybir as mybir
    from concourse import tile

    OP = mybir.AluOpType
    AF = mybir.ActivationFunctionType
    BF16 = mybir.dt.bfloat16
    F32 = mybir.dt.float32
    F8 = mybir.dt.float8e4
    DR = mybir.MatmulPerfMode.DoubleRow

    nc = bacc.Bacc(
        "TRN2",
        target_bir_lowering=False,
        debug=False,
        enable_asserts=False,
        num_devices=n_cores,
    )

    cfg = []
    for li, (Cin, Cout, k, stride, pad, Hi, Wi, Ho, Wo) in enumerate(shapes):
        CB = (Cin + 127) // 128
        OB = (Cout + 127) // 128
        KB = CB * k * k
        M = N_LOC * Ho * Wo
        Mpad = (M + 15) // 16 * 16
        cfg.append(
            dict(
                li=li, Cin=Cin, Cout=Cout, k=k, stride=stride, pad=pad,
                Hi=Hi, Wi=Wi, Ho=Ho, Wo=Wo, CB=CB, OB=OB, KB=KB, M=M,
                Mpad=Mpad, NM=n_total * Ho * Wo, L=LEVELS[li],
            )
        )

    # ---------------- DRAM I/O ----------------
    L1 = cfg[0]
    xt_d = nc.dram_tensor("xt", [L1["CB"], 128, N_LOC * L1["Hi"] * L1["Wi"]], BF16,
                          kind="ExternalInput")
    w_d, g_d, b_d, thr_d = [], [], [], []
    for L in cfg:
        li = L["li"] + 1
        vdt = F8 if FP8[L["li"]] else BF16
        w_d.append(nc.dram_tensor(f"w{li}q", [L["KB"] * L["L"], 128, L["Cout"]],
                                  vdt, kind="ExternalInput"))
        thr_d.append(nc.dram_tensor(f"thr{li}", [128, 2 * L["L"] + 2], F32,
                                    kind="ExternalInput"))
        g_d.append(nc.dram_tensor(f"g{li}", [128, L["OB"]], F32, kind="ExternalInput"))
        b_d.append(nc.dram_tensor(f"b{li}", [128, L["OB"]], F32, kind="ExternalInput"))
    LL = cfg[-1]
    out_d = nc.dram_tensor("out", [N_LOC, LL["Cout"], LL["Ho"], LL["Wo"]], F32,
                           kind="ExternalOutput")

    # ---------------- persistent SBUF ----------------
    def sb(name, cols, dt):
        return nc.alloc_sbuf_tensor(name, [128, cols], dt)

    xsb = sb("xsb", L1["CB"] * N_LOC * L1["Hi"] * L1["Wi"], BF16)

    act = []
    act.append(sb("act1p", cfg[0]["OB"] * N_LOC * 2 * 2 * 20 * 20, BF16))
    act.append(sb("act2", cfg[1]["OB"] * N_LOC * 19 * 19, BF16))
    act.append(sb("act3p", N_LOC * 2 * 2 * 11 * 12, BF16))
    act.append(sb("act4", cfg[3]["OB"] * N_LOC * 10 * 10, BF16))
    act.append(sb("act5p", N_LOC * 2 * 2 * 5 * 6, BF16))
    act.append(sb("fin6", cfg[5]["OB"] * N_LOC * 4 * 4, F32))

    S_max = max(L["OB"] * L["M"] for L in cfg)
    S = sb("S", S_max, F32)

    thr_sb = [sb(f"thrsb{L['li']}", 2 * L["L"] + 2, F32) for L in cfg]
    g_sb = [sb(f"gsb{L['li']}", L["OB"], F32) for L in cfg]
    b_sb = [sb(f"bsb{L['li']}", L["OB"], F32) for L in cfg]

    # f32 ones for the G pathway
    ones1f = sb("ones1f", 1, F32)       # [128,1] column (reduce lhsT)
    onesrf = sb("onesrf", 128, F32)     # row 0 used as [1,128] (bcast lhsT)
    # G accumulation + g' row
    Mmax = max(L["M"] for L in cfg)
    gacc = sb("gacc", Mmax, F32)
    gtmp = sb("gtmp", Mmax, F32)
    grow = sb("grow", Mmax, F32)        # g' = -G/(2*Delta), row 0

    stats_sb, stats_g, abwork = {}, {}, {}
    sqfull = sb("sqfull", SQW, F32)
    spart = sb("spart", 8, F32)

    with tile.TileContext(nc) as tc:
        # ---------------- input loads ----------------
        for cb in range(L1["CB"]):
            W0 = N_LOC * L1["Hi"] * L1["Wi"]
            nc.sync.dma_start(xsb.ap()[:, cb * W0:(cb + 1) * W0], xt_d.ap()[cb])
        for L in cfg:
            li = L["li"]
            nc.sync.dma_start(thr_sb[li].ap(), thr_d[li].ap())
            nc.sync.dma_start(g_sb[li].ap(), g_d[li].ap())
            nc.sync.dma_start(b_sb[li].ap(), b_d[li].ap())
        nc.vector.memset(ones1f.ap(), 1.0)
        nc.vector.memset(onesrf.ap(), 1.0)
        nc.gpsimd.memset(act[0].ap(), 0.0)
        nc.gpsimd.memset(act[2].ap(), 0.0)
        nc.gpsimd.memset(act[4].ap(), 0.0)

        with (
            tc.tile_pool(name="u", bufs=4) as up,
            tc.tile_pool(name="u8", bufs=6) as u8p,
            tc.tile_pool(name="v", bufs=6) as vp,
            tc.tile_pool(name="ps", bufs=8, space="PSUM") as psp,
            tc.tile_pool(name="dram", bufs=2, space="DRAM") as dramp,
        ):
            # ============ source window AP per (layer, kb[, piece]) ========
            def src_window(L, kb, n=None):
                """Window for tap kb; n=None -> full M, else image-piece n."""
                li = L["li"]
                k = L["k"]
                M = L["M"]
                if li == 0:
                    W0 = N_LOC * L["Hi"] * L["Wi"]
                    v = xsb.ap()[:, kb * W0:(kb + 1) * W0]
                    if n is None:
                        return v
                    poff, chks = PIECES[0][n]
                    return v[:, poff:poff + sum(chks)]
                if k == 1:
                    src = act[li - 1]
                    return src.ap()[:, kb * M:(kb + 1) * M]
                if li == 1:
                    src, CBv, R, C = act[0], L["CB"], 20, 20
                elif li == 3:
                    src, CBv, R, C = act[2], 1, 11, 12
                else:
                    src, CBv, R, C = act[4], 1, 5, 6
                Ho, Wo, pad = L["Ho"], L["Wo"], L["pad"]
                if CBv > 1:
                    v = src.ap().rearrange(
                        "p (cb n ip jp r c) -> p cb n ip jp r c",
                        cb=CBv, n=N_LOC, ip=2, jp=2, r=R, c=C)
                else:
                    v = src.ap().rearrange(
                        "p (n ip jp r c) -> p n ip jp r c",
                        n=N_LOC, ip=2, jp=2, r=R, c=C)

                def sel(d):
                    if pad == 1:
                        return (1, 0, 0) if d == 0 else ((0, 0, 0) if d == 1 else (1, 1, 1))
                    return (0, 0, 0) if d == 0 else ((1, 0, 0) if d == 1 else (0, 1, 1))

                cb, r9 = divmod(kb, 9)
                dy, dx = divmod(r9, 3)
                ipv, rs, _ = sel(dy)
                jpv, _, cs = sel(dx)
                if CBv > 1:
                    w = v[:, cb, :, ipv, jpv, rs:rs + Ho, cs:cs + Wo]
                else:
                    w = v[:, :, ipv, jpv, rs:rs + Ho, cs:cs + Wo]
                if n is None:
                    return w
                return w[:, n]

            # deferred BN-applies of the previous layer: (apply_fn, ob).
            # Popped at the next layer's G-phase just before the first read
            # of the corresponding act channel-block, hiding the AllReduce
            # round-trip behind the G/production work of earlier blocks.
            pending = []

            def pop_applies(upto_ob):
                while pending and pending[0][1] <= upto_ob:
                    fa, ob_prev = pending.pop(0)
                    fa(ob_prev)

            # ============ conv layers ============
            for L in cfg:
                li, Cout, KB, M, Mpad, OB, Llv = (
                    L["li"], L["Cout"], L["KB"], L["M"], L["Mpad"], L["OB"],
                    L["L"])
                stats_sb[li] = sb(f"stats{li}", 2 * OB, F32)
                stats_g[li] = sb(f"statsg{li}", 2 * OB, F32)
                abwork[li] = sb(f"abw{li}", 12 * OB, F32)

                Ho, Wo = L["Ho"], L["Wo"]
                npieces = len(PIECES[li])

                def finish_stats(ob, li=li, L=L, M=M, OB=OB):
                    # stats (in <=2 pieces) + AllReduce launch
                    npiece = (M + SQW - 1) // SQW
                    for h in range(npiece):
                        h0 = h * SQW
                        hN = min(SQW, M - h0)
                        src = S.ap()[:, ob * M + h0:ob * M + h0 + hN]
                        nc.vector.tensor_scalar(
                            sqfull.ap()[:, :hN], src, 0.0, 0.0,
                            op0=OP.add, op1=OP.add,
                            accum_out=spart.ap()[:, h:h + 1],
                        )
                        nc.scalar.activation(
                            sqfull.ap()[:, :hN], src, AF.Square,
                            accum_out=spart.ap()[:, 4 + h:5 + h],
                        )
                    if npiece == 1:
                        nc.vector.tensor_scalar_add(
                            stats_sb[li].ap()[:, 2 * ob:2 * ob + 1],
                            spart.ap()[:, 0:1], 0.0)
                        nc.vector.tensor_scalar_add(
                            stats_sb[li].ap()[:, 2 * ob + 1:2 * ob + 2],
                            spart.ap()[:, 4:5], 0.0)
                    else:
                        nc.vector.tensor_tensor(
                            stats_sb[li].ap()[:, 2 * ob:2 * ob + 1],
                            spart.ap()[:, 0:1], spart.ap()[:, 1:2],
                            op=OP.add)
                        nc.vector.tensor_tensor(
                            stats_sb[li].ap()[:, 2 * ob + 1:2 * ob + 2],
                            spart.ap()[:, 4:5], spart.ap()[:, 5:6],
                            op=OP.add)
                    # stats DMAs + collective all ride the gpsimd (Pool)
                    # queue so the AllReduce wait never blocks the V-tile
                    # prefetch stream on the SP queue.
                    sti = dramp.tile([128, 2], F32, tag=f"sti{li}_{ob}",
                                     name=f"sti{li}_{ob}")
                    sto = dramp.tile([128, 2], F32, tag=f"sto{li}_{ob}",
                                     name=f"sto{li}_{ob}")
                    nc.gpsimd.dma_start(sti[:, :],
                                        stats_sb[li].ap()[:, 2 * ob:2 * ob + 2])
                    if mock_cc:
                        nc.gpsimd.dma_start(sto[:, :], sti[:, :])
                    else:
                        nc.gpsimd.collective_compute(
                            "AllReduce", OP.add,
                            replica_groups=[list(range(n_cores))],
                            ins=[sti.opt()], outs=[sto.opt()],
                        )
                    nc.gpsimd.dma_start(stats_g[li].ap()[:, 2 * ob:2 * ob + 2],
                                        sto[:, :])

                def finish_apply(ob, li=li, L=L, M=M, OB=OB, Ho=Ho, Wo=Wo):
                    aw = abwork[li].ap()

                    def col(i):
                        return aw[:, i * OB + ob:i * OB + ob + 1]

                    (mean, ex2, m2, vpe, sq_, rc, u, s2, rinv, ga, _unused,
                     Bv) = [col(i) for i in range(12)]
                    inm = 1.0 / L["NM"]
                    nc.vector.tensor_scalar_mul(
                        mean, stats_g[li].ap()[:, 2 * ob:2 * ob + 1], inm)
                    nc.vector.tensor_scalar_mul(
                        ex2, stats_g[li].ap()[:, 2 * ob + 1:2 * ob + 2], inm)
                    nc.scalar.activation(m2, mean, AF.Square)
                    nc.vector.tensor_tensor(vpe, ex2, m2, op=OP.subtract)
                    nc.vector.tensor_scalar_add(vpe, vpe, EPS)
                    nc.scalar.activation(sq_, vpe, AF.Sqrt)
                    nc.vector.reciprocal(rc, sq_)
                    nc.vector.tensor_tensor(u, vpe, rc, op=OP.mult)
                    nc.vector.tensor_tensor(s2, sq_, u, op=OP.add)
                    nc.vector.tensor_scalar_mul(s2, s2, 0.5)
                    nc.vector.reciprocal(rinv, s2)
                    gcol = g_sb[li].ap()[:, ob:ob + 1]
                    bcol = b_sb[li].ap()[:, ob:ob + 1]
                    nc.vector.tensor_tensor(ga, gcol, rinv, op=OP.mult)
                    # y = A*S + (b - A*mean)
                    nc.vector.tensor_tensor(Bv, mean, ga, op=OP.mult)
                    nc.vector.tensor_tensor(Bv, bcol, Bv, op=OP.subtract)

                    def apply_tsp(dst_ap, src_ap):
                        nc.vector.tensor_scalar(
                            dst_ap, src_ap, ga, Bv, OP.mult, OP.add)

                    if li in (0, 2, 4):
                        if li == 0:
                            R, C = 20, 20
                            dstv = act[0].ap().rearrange(
                                "p (obb n ip jp r c) -> p obb n ip jp r c",
                                obb=OB, n=N_LOC, ip=2, jp=2, r=R, c=C)
                        elif li == 2:
                            R, C = 11, 12
                            dstv = act[2].ap().rearrange(
                                "p (n ip jp r c) -> p n ip jp r c",
                                n=N_LOC, ip=2, jp=2, r=R, c=C)
                        else:
                            R, C = 5, 6
                            dstv = act[4].ap().rearrange(
                                "p (n ip jp r c) -> p n ip jp r c",
                                n=N_LOC, ip=2, jp=2, r=R, c=C)
                        Sv = S.ap()[:, :OB * M].rearrange(
                            "p (obb n i j) -> p obb n i j",
                            obb=OB, n=N_LOC, i=Ho, j=Wo)
                        pad = 1 if li in (0, 2) else 0
                        for bpar in (0, 1):
                            for dpar in (0, 1):
                                srcv = Sv[:, ob, :, bpar::2, dpar::2]
                                na, ncc = srcv.shape[2], srcv.shape[3]
                                if pad == 1:
                                    ipv, rs = (0, 0) if bpar == 0 else (1, 1)
                                    jpv, cs = (0, 0) if dpar == 0 else (1, 1)
                                else:
                                    ipv, rs = (0, 0) if bpar == 0 else (1, 0)
                                    jpv, cs = (0, 0) if dpar == 0 else (1, 0)
                                if li == 0:
                                    dst = dstv[:, ob, :, ipv, jpv,
                                               rs:rs + na, cs:cs + ncc]
                                else:
                                    dst = dstv[:, :, ipv, jpv,
                                               rs:rs + na, cs:cs + ncc]
                                apply_tsp(dst, srcv)
                        blk = act[li].ap().shape[1] // OB
                        tgt = act[li].ap()[:, ob * blk:(ob + 1) * blk]
                        nc.vector.tensor_scalar(tgt, tgt, 0.0, 6.0,
                                                OP.max, OP.min)
                    else:
                        dst_t = act[li] if li != 5 else act[5]
                        apply_tsp(dst_t.ap()[:, ob * M:(ob + 1) * M],
                                  S.ap()[:, ob * M:(ob + 1) * M])
                        tgt = dst_t.ap()[:, ob * M:(ob + 1) * M]
                        nc.vector.tensor_scalar(tgt, tgt, 0.0, 6.0,
                                                OP.max, OP.min)

                # thresholds
                tcol = lambda l: thr_sb[li].ap()[:, l:l + 1]
                ntcol = lambda l: thr_sb[li].ap()[:, Llv + l:Llv + l + 1]
                t0col = thr_sb[li].ap()[:, 2 * Llv:2 * Llv + 1]
                dcol = thr_sb[li].ap()[0:1, 2 * Llv + 1:2 * Llv + 2]

                kk = L["k"] * L["k"]

                for pi, (poff, chunks) in enumerate(PIECES[li]):
                    n_arg = pi if npieces > 1 else None
                    Mp = sum(chunks)
                    Mp_pad = (Mp + 15) // 16 * 16
                    coff = [sum(chunks[:c]) for c in range(len(chunks))]
                    nchunk = len(chunks)

                    # ---- PSUM tiles: G slots reserved first, then conv ----
                    fp8 = FP8[li]
                    gps = [psp.tile([128, chunks[c]], F32, tag="ps",
                                    name=f"gps{li}_{pi}_{c}",
                                    padded_shape=[128, 512])
                           for c in range(nchunk)]
                    pss = {}
                    for ob in range(OB):
                        for c in range(nchunk):
                            pss[(ob, c)] = psp.tile(
                                [128, chunks[c]], F32, tag="ps",
                                name=f"ps{li}_{pi}_{ob}_{c}",
                                padded_shape=[128, 512])

                    GV = 8  # levels per V DMA (HWDGE fixed cost is per instr)
                    prod_ctr = 0
                    for kb in range(KB):
                        # previous layer's BN-apply for channel-block cb must
                        # be issued before any read of that act block
                        pop_applies(kb // kk)
                        win = src_window(L, kb, n_arg)
                        # G accumulation interleaved with production so the
                        # serial abs->add chain never head-blocks the queues
                        if kb == 0:
                            nc.scalar.activation(
                                gacc.ap()[:, :Mp], win, AF.Abs,
                                bias=t0col, scale=1.0)
                        else:
                            nc.scalar.activation(
                                gtmp.ap()[:, :Mp], win, AF.Abs,
                                bias=t0col, scale=1.0)
                            nc.vector.tensor_tensor(
                                gacc.ap()[:, :Mp], gacc.ap()[:, :Mp],
                                gtmp.ap()[:, :Mp], op=OP.add)
                        for g0 in range(0, Llv, GV):
                            gN = min(GV, Llv - g0)
                            vdt = F8 if fp8 else BF16
                            vt = vp.tile([128, gN * Cout], vdt, tag="v")
                            nc.sync.dma_start(
                                vt.rearrange("p (g c) -> p g c", g=gN),
                                w_d[li].ap()[kb * Llv + g0:
                                             kb * Llv + g0 + gN].rearrange(
                                    "g p c -> p g c"))
                            vv = vt.rearrange("p (g c) -> p g c", g=gN)
                            if fp8:
                                # level pairs -> DoubleRow matmuls (128 rows)
                                for i2 in range(0, gN, 2):
                                    scr = u8p.tile([128, 2 * Mp_pad], F8,
                                                   tag="u8")
                                    for t in (0, 1):
                                        l = g0 + i2 + t
                                        dst = scr[:, t * Mp_pad:
                                                  t * Mp_pad + Mp]
                                        p = PRODPAT[(kb * Llv + l) % 16]
                                        if p == "A":
                                            nc.scalar.activation(
                                                dst, win, AF.Sign,
                                                bias=ntcol(l), scale=1.0)
                                        else:
                                            eng = (nc.gpsimd if p == "P"
                                                   else nc.vector)
                                            eng.tensor_scalar(
                                                dst, win, tcol(l), None,
                                                op0=OP.is_gt)
                                    sv = scr.rearrange("p (t m) -> p t m",
                                                       t=2)
                                    first = (kb == 0 and g0 + i2 == 0)
                                    for ob in range(OB):
                                        lhsT = vv[:, i2:i2 + 2,
                                                  ob * 128:(ob + 1) * 128]
                                        for c in range(nchunk):
                                            c0, cN = coff[c], chunks[c]
                                            nc.tensor.matmul(
                                                pss[(ob, c)][:, :cN],
                                                lhsT,
                                                sv[:, :, c0:c0 + cN],
                                                start=first, stop=False,
                                                perf_mode=DR,
                                                skip_group_check=True)
                            else:
                                for l in range(g0, g0 + gN):
                                    vo = (l - g0) * Cout
                                    ut = up.tile([128, Mp_pad], BF16,
                                                 tag="u")
                                    nc.vector.tensor_scalar(
                                        ut[:, :Mp], win, tcol(l), None,
                                        op0=OP.is_gt)
                                    first = (kb == 0 and l == 0)
                                    for ob in range(OB):
                                        for c in range(nchunk):
                                            c0, cN = coff[c], chunks[c]
                                            nc.tensor.matmul(
                                                pss[(ob, c)][:, :cN],
                                                vt[:, vo + ob * 128:
                                                   vo + (ob + 1) * 128],
                                                ut[:, c0:c0 + cN],
                                                start=first, stop=False,
                                                skip_group_check=True)

                    # ---- G reduce to row + scale to g' (per chunk) ----
                    for c in range(nchunk):
                        c0, cN = coff[c], chunks[c]
                        nc.tensor.matmul(
                            gps[c][0:1, :cN], ones1f.ap(),
                            gacc.ap()[:, c0:c0 + cN],
                            start=True, stop=True, skip_group_check=True)
                        nc.vector.tensor_scalar_mul(
                            grow.ap()[0:1, poff + c0:poff + c0 + cN],
                            gps[c][0:1, :cN], dcol)

                    # ---- broadcast g' into all psum tiles, evacuate ----
                    for ob in range(OB):
                        for c in range(nchunk):
                            c0, cN = coff[c], chunks[c]
                            nc.tensor.matmul(
                                pss[(ob, c)][:, :cN], onesrf.ap()[0:1, :],
                                grow.ap()[0:1, poff + c0:poff + c0 + cN],
                                start=False, stop=True,
                                skip_group_check=True)
                            # evacuate on Act (Identity) to keep DVE free
                            nc.scalar.activation(
                                S.ap()[:, ob * M + poff + c0:
                                       ob * M + poff + c0 + cN],
                                pss[(ob, c)][:, :cN], AF.Identity)
                        if pi == npieces - 1:
                            finish_stats(ob)
                            pending.append((finish_apply, ob))

            for fa, ob_prev in pending:
                fa(ob_prev)
            pending.clear()

            # ---------------- final output DMA ----------------
            hw = LL["Ho"] * LL["Wo"]
            finv = act[5].ap().rearrange("p (ob n hw) -> p ob n hw",
                                         ob=LL["OB"], n=N_LOC, hw=hw)
            dst = out_d.ap().rearrange("n (ob p) h w -> p ob n (h w)",
                                       ob=LL["OB"], p=128)
            for ob in range(LL["OB"]):
                nc.sync.dma_start(dst[:, ob], finv[:, ob])

    nc.compile()
    return nc


def _get_nc():
    if "nc" not in _NC_CACHE:
        _NC_CACHE["nc"] = _build_nc()
    return _NC_CACHE["nc"]


def kernel(**inputs) -> np.ndarray:
    import time as _time
    from concourse.bass_utils import run_bass_kernel_spmd

    nc = _get_nc()
    in_maps = _host_prep(inputs)
    last = None
    for attempt in range(3):
        try:
            res = run_bass_kernel_spmd(nc, in_maps, core_ids=list(range(N_CORES)))
            outs = [np.asarray(r["out"]).reshape(N_LOC, 256, 4, 4)
                    for r in res.results]
            return np.concatenate(outs, axis=0).astype(np.float32)
        except Exception as e:  # transient axon mesh desync: wait + retry
            last = e
            _time.sleep(20 * (attempt + 1))
    raise last


if __name__ == "__main__":
    nc = _build_nc()
    print("build + compile OK")


# revision 41
# speedup vs baseline: 56.8027x; 1.2026x over previous
"""AdderNet CNN (6x adder_conv + sync-BN + ReLU6) on 8 Trainium2 NeuronCores.

v4: thermometer-quantization.  |x-w| = x + w - 2*min(x,w), and
min(x,w) - t0 = integral of 1[t<x]*1[t<w] over the weight range
~= sum_l Delta * u_l(x) * v_l(w) with u_l = 1[x > t_l], v_l = 1[w > t_l]
on a midpoint grid t_l = t0 + (l+0.5)*Delta covering only the (clipped)
weight range.  This is a DENSE matmul over binary features with
contraction dim 128*L per tap -- one lhsT [128, Cout] per (tap, level)
covers ALL output channels, replacing the per-channel one-hot matmuls:

  S_o(m) = P_o(m) + g'(m) (+ per-channel const, which BN cancels; BN is
  also per-channel scale-invariant so the 2*Delta factor drops out)
  P[o,m] = sum_{kb,l} V[kb,l,:,o] . u[kb,l,:,m]   (integer counts, exact)
  g'(m)  = -G(m)/(2*Delta),  G(m) = sum_k |x_k(m) - t0|  (f32 pathway)

Quantization error for taps with x > w depends only on w -> per-channel
constant -> cancelled by BN.  Data-dependent noise only from taps with
x inside the tiny clipped weight range; grid is 0-aligned so the
post-ReLU6 point mass at x=0 is exact.  Weights are clipped to
+-3 sigma (the clip residual is again a BN-cancelled constant).

Per layer: levels L = [48,16,16,16,16,16]; binary V tiles (bf16) are
streamed from DRAM; u tiles produced on DVE (4x mode, 0.26ns/col);
G accumulated in f32 (Act abs + DVE adds, one f32 ones-matmul reduce
per chunk, f32 ones-broadcast back into PSUM).  BN stats/apply and the
tiny per-ob (sum,sumsq) AllReduce are unchanged from v3.

Sharding: data-parallel over batch (2 images/core), sync-BN via
AllReduce of per-channel (sum, sumsq) per layer.
"""

import sys
import numpy as np

if "/opt/trn_rl_repo" not in sys.path:
    sys.path.insert(0, "/opt/trn_rl_repo")

import ml_dtypes

N_CORES = 8
N_LOC = 2  # images per core
EPS = 1e-5

# (Cin, Cout, k, stride, pad, Hi, Wi, Ho, Wo)
LAYER_SHAPES = [
    (512, 256, 1, 1, 0, 38, 38, 38, 38),
    (256, 512, 3, 2, 1, 38, 38, 19, 19),
    (512, 128, 1, 1, 0, 19, 19, 19, 19),
    (128, 256, 3, 2, 1, 19, 19, 10, 10),
    (256, 128, 1, 1, 0, 10, 10, 10, 10),
    (128, 256, 3, 2, 0, 10, 10, 4, 4),
]

# thermometer levels per layer (even) and weight-clip in sigmas
LEVELS = [36, 12, 12, 12, 12, 12]
WCLIP = 3.0
# layers using fp8 DoubleRow matmuls (PE at 2x; u tiles + V in fp8)
FP8 = [False, False, False, False, False, False]
# producer assignment per (kb*L + l) % 16 for fp8 layers.  D=DVE is_gt
# {0,1}, P=Pool is_gt {0,1}, A=Act Sign {-1,+1} with V scaled by 0.5 on
# the host (the -0.5*sum(V) constant is per-channel -> BN cancels it).
PRODPAT = "DADPDADDADPDADDP"

# per-layer piece/chunk structure: list of (piece_col_offset, [chunk sizes])
# pieces split M so that the live PSUM tiles fit the 8-bank budget
# (fp8-DR doubles the tile count: DR writes land in rows 0:64 only).
# L1 splits by image half, L2 by image n.
PIECES = [
    [(0, [481, 481]), (962, [481, 481]), (1924, [482, 482])],
    [(0, [361]), (361, [361])],
    [(0, [361, 361])],
    [(0, [200])],
    [(0, [200])],
    [(0, [32])],
]

SQW = 1456  # f32 stats scratch cols (stats computed in <=2 pieces)

_NC_CACHE = {}


def _grid(w, L):
    """Midpoint grid over the clipped weight range, 0-aligned.
    Returns (t0, Delta, thresholds[L])."""
    c = float(min(np.abs(w).max() * 1.0001, WCLIP * w.std()))
    j0 = L // 2
    Delta = c / j0
    t0 = -j0 * Delta
    t = t0 + (np.arange(L, dtype=np.float64) + 0.5) * Delta
    return t0, Delta, t.astype(np.float32)


def _host_prep(inputs, shapes=LAYER_SHAPES, n_total=16):
    """Build per-core in_maps from the raw reference inputs."""
    bf16 = ml_dtypes.bfloat16
    x = np.asarray(inputs["x"], dtype=np.float32)
    maps_common = {}
    for li, (Cin, Cout, k, *_rest) in enumerate(shapes, start=1):
        w = np.asarray(inputs[f"w{li}"], dtype=np.float32)  # [Cout,Cin,k,k]
        CB = (Cin + 127) // 128
        if k == 1:
            wt = w[:, :, 0, 0].T.reshape(CB, 128, Cout)
        else:
            # kb = cb*9 + dy*3 + dx
            wt = (
                w.transpose(1, 2, 3, 0)  # [Cin, k, k, Cout]
                .reshape(CB, 128, k * k, Cout)
                .transpose(0, 2, 1, 3)  # [CB, k*k, 128, Cout]
                .reshape(CB * k * k, 128, Cout)
            )
        L = LEVELS[li - 1]
        t0, Delta, t = _grid(wt, L)
        # V[kb*L + l, c, o] = 1[w > t_l], binary (fp8 for DR layers)
        KB = wt.shape[0]
        V = (wt[:, None, :, :] > t[None, :, None, None])  # [KB, L, 128, Cout]
        vdt = ml_dtypes.float8_e4m3 if FP8[li - 1] else bf16
        Vf = V.reshape(KB * L, 128, Cout).astype(np.float32)
        if FP8[li - 1]:
            for kl in range(KB * L):
                if PRODPAT[kl % 16] == "A":
                    Vf[kl] *= 0.5
        maps_common[f"w{li}q"] = np.ascontiguousarray(Vf.astype(vdt))
        thr = np.zeros((128, 2 * L + 2), np.float32)
        thr[:, :L] = t[None, :]
        thr[:, L:2 * L] = -t[None, :]
        thr[:, 2 * L] = -t0
        thr[:, 2 * L + 1] = -1.0 / (2.0 * Delta)
        maps_common[f"thr{li}"] = thr
        OB = (Cout + 127) // 128
        g = np.asarray(inputs[f"g{li}"], dtype=np.float32).reshape(OB, 128).T
        b = np.asarray(inputs[f"b{li}"], dtype=np.float32).reshape(OB, 128).T
        maps_common[f"g{li}"] = np.ascontiguousarray(g)
        maps_common[f"b{li}"] = np.ascontiguousarray(b)

    Cin0, _, _, _, _, Hi, Wi, _, _ = shapes[0]
    CB0 = (Cin0 + 127) // 128
    in_maps = []
    n_cores = n_total // N_LOC
    for c in range(n_cores):
        xs = x[c * N_LOC : (c + 1) * N_LOC]  # [2, Cin, H, W]
        xt = (
            xs.transpose(1, 0, 2, 3)
            .reshape(CB0, 128, N_LOC * Hi * Wi)
            .astype(bf16)
        )
        m = dict(maps_common)
        m["xt"] = np.ascontiguousarray(xt)
        in_maps.append(m)
    return in_maps


def _build_nc(shapes=LAYER_SHAPES, n_total=16, n_cores=N_CORES, mock_cc=False):
    import concourse.bacc as bacc
    import concourse.mybir as mybir
    from concourse import tile

    OP = mybir.AluOpType
    AF = mybir.ActivationFunctionType
    BF16 = mybir.dt.bfloat16
    F32 = mybir.dt.float32
    F8 = mybir.dt.float8e4
    DR = mybir.MatmulPerfMode.DoubleRow

    nc = bacc.Bacc(
        "TRN2",
        target_bir_lowering=False,
        debug=False,
        enable_asserts=False,
        num_devices=n_cores,
    )

    cfg = []
    for li, (Cin, Cout, k, stride, pad, Hi, Wi, Ho, Wo) in enumerate(shapes):
        CB = (Cin + 127) // 128
        OB = (Cout + 127) // 128
        KB = CB * k * k
        M = N_LOC * Ho * Wo
        Mpad = (M + 15) // 16 * 16
        cfg.append(
            dict(
                li=li, Cin=Cin, Cout=Cout, k=k, stride=stride, pad=pad,
                Hi=Hi, Wi=Wi, Ho=Ho, Wo=Wo, CB=CB, OB=OB, KB=KB, M=M,
                Mpad=Mpad, NM=n_total * Ho * Wo, L=LEVELS[li],
            )
        )

    # ---------------- DRAM I/O ----------------
    L1 = cfg[0]
    xt_d = nc.dram_tensor("xt", [L1["CB"], 128, N_LOC * L1["Hi"] * L1["Wi"]], BF16,
                          kind="ExternalInput")
    w_d, g_d, b_d, thr_d = [], [], [], []
    for L in cfg:
        li = L["li"] + 1
        vdt = F8 if FP8[L["li"]] else BF16
        w_d.append(nc.dram_tensor(f"w{li}q", [L["KB"] * L["L"], 128, L["Cout"]],
                                  vdt, kind="ExternalInput"))
        thr_d.append(nc.dram_tensor(f"thr{li}", [128, 2 * L["L"] + 2], F32,
                                    kind="ExternalInput"))
        g_d.append(nc.dram_tensor(f"g{li}", [128, L["OB"]], F32, kind="ExternalInput"))
        b_d.append(nc.dram_tensor(f"b{li}", [128, L["OB"]], F32, kind="ExternalInput"))
    LL = cfg[-1]
    out_d = nc.dram_tensor("out", [N_LOC, LL["Cout"], LL["Ho"], LL["Wo"]], F32,
                           kind="ExternalOutput")

    # ---------------- persistent SBUF ----------------
    def sb(name, cols, dt):
        return nc.alloc_sbuf_tensor(name, [128, cols], dt)

    xsb = sb("xsb", L1["CB"] * N_LOC * L1["Hi"] * L1["Wi"], BF16)

    act = []
    act.append(sb("act1p", cfg[0]["OB"] * N_LOC * 2 * 2 * 20 * 20, BF16))
    act.append(sb("act2", cfg[1]["OB"] * N_LOC * 19 * 19, BF16))
    act.append(sb("act3p", N_LOC * 2 * 2 * 11 * 12, BF16))
    act.append(sb("act4", cfg[3]["OB"] * N_LOC * 10 * 10, BF16))
    act.append(sb("act5p", N_LOC * 2 * 2 * 5 * 6, BF16))
    act.append(sb("fin6", cfg[5]["OB"] * N_LOC * 4 * 4, F32))

    S_max = max(L["OB"] * L["M"] for L in cfg)
    S = sb("S", S_max, F32)

    thr_sb = [sb(f"thrsb{L['li']}", 2 * L["L"] + 2, F32) for L in cfg]
    g_sb = [sb(f"gsb{L['li']}", L["OB"], F32) for L in cfg]
    b_sb = [sb(f"bsb{L['li']}", L["OB"], F32) for L in cfg]

    # f32 ones for the G pathway
    ones1f = sb("ones1f", 1, F32)       # [128,1] column (reduce lhsT)
    onesrf = sb("onesrf", 128, F32)     # row 0 used as [1,128] (bcast lhsT)
    # G accumulation + g' row
    Mmax = max(L["M"] for L in cfg)
    gacc = sb("gacc", Mmax, F32)
    gtmp = sb("gtmp", Mmax, F32)
    grow = sb("grow", Mmax, F32)        # g' = -G/(2*Delta), row 0

    stats_sb, stats_g, abwork = {}, {}, {}
    sqfull = sb("sqfull", SQW, F32)
    spart = sb("spart", 8, F32)

    with tile.TileContext(nc) as tc:
        # ---------------- input loads ----------------
        for cb in range(L1["CB"]):
            W0 = N_LOC * L1["Hi"] * L1["Wi"]
            nc.sync.dma_start(xsb.ap()[:, cb * W0:(cb + 1) * W0], xt_d.ap()[cb])
        for L in cfg:
            li = L["li"]
            nc.sync.dma_start(thr_sb[li].ap(), thr_d[li].ap())
            nc.sync.dma_start(g_sb[li].ap(), g_d[li].ap())
            nc.sync.dma_start(b_sb[li].ap(), b_d[li].ap())
        nc.vector.memset(ones1f.ap(), 1.0)
        nc.vector.memset(onesrf.ap(), 1.0)
        nc.gpsimd.memset(act[0].ap(), 0.0)
        nc.gpsimd.memset(act[2].ap(), 0.0)
        nc.gpsimd.memset(act[4].ap(), 0.0)

        with (
            tc.tile_pool(name="u", bufs=4) as up,
            tc.tile_pool(name="u8", bufs=6) as u8p,
            tc.tile_pool(name="v", bufs=6) as vp,
            tc.tile_pool(name="ps", bufs=8, space="PSUM") as psp,
            tc.tile_pool(name="dram", bufs=2, space="DRAM") as dramp,
        ):
            # ============ source window AP per (layer, kb[, piece]) ========
            def src_window(L, kb, n=None):
                """Window for tap kb; n=None -> full M, else image-piece n."""
                li = L["li"]
                k = L["k"]
                M = L["M"]
                if li == 0:
                    W0 = N_LOC * L["Hi"] * L["Wi"]
                    v = xsb.ap()[:, kb * W0:(kb + 1) * W0]
                    if n is None:
                        return v
                    poff, chks = PIECES[0][n]
                    return v[:, poff:poff + sum(chks)]
                if k == 1:
                    src = act[li - 1]
                    return src.ap()[:, kb * M:(kb + 1) * M]
                if li == 1:
                    src, CBv, R, C = act[0], L["CB"], 20, 20
                elif li == 3:
                    src, CBv, R, C = act[2], 1, 11, 12
                else:
                    src, CBv, R, C = act[4], 1, 5, 6
                Ho, Wo, pad = L["Ho"], L["Wo"], L["pad"]
                if CBv > 1:
                    v = src.ap().rearrange(
                        "p (cb n ip jp r c) -> p cb n ip jp r c",
                        cb=CBv, n=N_LOC, ip=2, jp=2, r=R, c=C)
                else:
                    v = src.ap().rearrange(
                        "p (n ip jp r c) -> p n ip jp r c",
                        n=N_LOC, ip=2, jp=2, r=R, c=C)

                def sel(d):
                    if pad == 1:
                        return (1, 0, 0) if d == 0 else ((0, 0, 0) if d == 1 else (1, 1, 1))
                    return (0, 0, 0) if d == 0 else ((1, 0, 0) if d == 1 else (0, 1, 1))

                cb, r9 = divmod(kb, 9)
                dy, dx = divmod(r9, 3)
                ipv, rs, _ = sel(dy)
                jpv, _, cs = sel(dx)
                if CBv > 1:
                    w = v[:, cb, :, ipv, jpv, rs:rs + Ho, cs:cs + Wo]
                else:
                    w = v[:, :, ipv, jpv, rs:rs + Ho, cs:cs + Wo]
                if n is None:
                    return w
                return w[:, n]

            # deferred BN-applies of the previous layer: (apply_fn, ob).
            # Popped at the next layer's G-phase just before the first read
            # of the corresponding act channel-block, hiding the AllReduce
            # round-trip behind the G/production work of earlier blocks.
            pending = []

            def pop_applies(upto_ob):
                while pending and pending[0][1] <= upto_ob:
                    fa, ob_prev = pending.pop(0)
                    fa(ob_prev)

            # ============ conv layers ============
            for L in cfg:
                li, Cout, KB, M, Mpad, OB, Llv = (
                    L["li"], L["Cout"], L["KB"], L["M"], L["Mpad"], L["OB"],
                    L["L"])
                stats_sb[li] = sb(f"stats{li}", 2 * OB, F32)
                stats_g[li] = sb(f"statsg{li}", 2 * OB, F32)
                abwork[li] = sb(f"abw{li}", 12 * OB, F32)

                Ho, Wo = L["Ho"], L["Wo"]
                npieces = len(PIECES[li])

                def finish_stats(ob, li=li, L=L, M=M, OB=OB):
                    # stats (in <=2 pieces) + AllReduce launch
                    npiece = (M + SQW - 1) // SQW
                    for h in range(npiece):
                        h0 = h * SQW
                        hN = min(SQW, M - h0)
                        src = S.ap()[:, ob * M + h0:ob * M + h0 + hN]
                        nc.vector.tensor_scalar(
                            sqfull.ap()[:, :hN], src, 0.0, 0.0,
                            op0=OP.add, op1=OP.add,
                            accum_out=spart.ap()[:, h:h + 1],
                        )
                        nc.scalar.activation(
                            sqfull.ap()[:, :hN], src, AF.Square,
                            accum_out=spart.ap()[:, 4 + h:5 + h],
                        )
                    if npiece == 1:
                        nc.vector.tensor_scalar_add(
                            stats_sb[li].ap()[:, 2 * ob:2 * ob + 1],
                            spart.ap()[:, 0:1], 0.0)
                        nc.vector.tensor_scalar_add(
                            stats_sb[li].ap()[:, 2 * ob + 1:2 * ob + 2],
                            spart.ap()[:, 4:5], 0.0)
                    else:
                        nc.vector.tensor_tensor(
                            stats_sb[li].ap()[:, 2 * ob:2 * ob + 1],
                            spart.ap()[:, 0:1], spart.ap()[:, 1:2],
                            op=OP.add)
                        nc.vector.tensor_tensor(
                            stats_sb[li].ap()[:, 2 * ob + 1:2 * ob + 2],
                            spart.ap()[:, 4:5], spart.ap()[:, 5:6],
                            op=OP.add)
                    if ob == OB - 1:
                        layer_allreduce()

                def layer_allreduce(li=li, OB=OB):
                    # one batched AllReduce per layer for all (sum, sumsq)
                    # columns; rides the gpsimd (Pool) queue so the wait
                    # never blocks the V-prefetch stream on the SP queue.
                    sti = dramp.tile([128, 2 * OB], F32, tag=f"sti{li}",
                                     name=f"sti{li}")
                    sto = dramp.tile([128, 2 * OB], F32, tag=f"sto{li}",
                                     name=f"sto{li}")
                    nc.gpsimd.dma_start(sti[:, :], stats_sb[li].ap())
                    if mock_cc:
                        nc.gpsimd.dma_start(sto[:, :], sti[:, :])
                    else:
                        nc.gpsimd.collective_compute(
                            "AllReduce", OP.add,
                            replica_groups=[list(range(n_cores))],
                            ins=[sti.opt()], outs=[sto.opt()],
                        )
                    nc.gpsimd.dma_start(stats_g[li].ap(), sto[:, :])

                def finish_apply(ob, li=li, L=L, M=M, OB=OB, Ho=Ho, Wo=Wo):
                    aw = abwork[li].ap()

                    def col(i):
                        return aw[:, i * OB + ob:i * OB + ob + 1]

                    (mean, ex2, m2, vpe, sq_, rc, u, s2, rinv, ga, _unused,
                     Bv) = [col(i) for i in range(12)]
                    inm = 1.0 / L["NM"]
                    nc.vector.tensor_scalar_mul(
                        mean, stats_g[li].ap()[:, 2 * ob:2 * ob + 1], inm)
                    nc.vector.tensor_scalar_mul(
                        ex2, stats_g[li].ap()[:, 2 * ob + 1:2 * ob + 2], inm)
                    nc.scalar.activation(m2, mean, AF.Square)
                    nc.vector.tensor_tensor(vpe, ex2, m2, op=OP.subtract)
                    nc.vector.tensor_scalar_add(vpe, vpe, EPS)
                    nc.scalar.activation(sq_, vpe, AF.Sqrt)
                    nc.vector.reciprocal(rc, sq_)
                    nc.vector.tensor_tensor(u, vpe, rc, op=OP.mult)
                    nc.vector.tensor_tensor(s2, sq_, u, op=OP.add)
                    nc.vector.tensor_scalar_mul(s2, s2, 0.5)
                    nc.vector.reciprocal(rinv, s2)
                    gcol = g_sb[li].ap()[:, ob:ob + 1]
                    bcol = b_sb[li].ap()[:, ob:ob + 1]
                    nc.vector.tensor_tensor(ga, gcol, rinv, op=OP.mult)
                    # y = A*S + (b - A*mean)
                    nc.vector.tensor_tensor(Bv, mean, ga, op=OP.mult)
                    nc.vector.tensor_tensor(Bv, bcol, Bv, op=OP.subtract)

                    def apply_tsp(dst_ap, src_ap):
                        nc.vector.tensor_scalar(
                            dst_ap, src_ap, ga, Bv, OP.mult, OP.add)

                    if li in (0, 2, 4):
                        if li == 0:
                            R, C = 20, 20
                            dstv = act[0].ap().rearrange(
                                "p (obb n ip jp r c) -> p obb n ip jp r c",
                                obb=OB, n=N_LOC, ip=2, jp=2, r=R, c=C)
                        elif li == 2:
                            R, C = 11, 12
                            dstv = act[2].ap().rearrange(
                                "p (n ip jp r c) -> p n ip jp r c",
                                n=N_LOC, ip=2, jp=2, r=R, c=C)
                        else:
                            R, C = 5, 6
                            dstv = act[4].ap().rearrange(
                                "p (n ip jp r c) -> p n ip jp r c",
                                n=N_LOC, ip=2, jp=2, r=R, c=C)
                        Sv = S.ap()[:, :OB * M].rearrange(
                            "p (obb n i j) -> p obb n i j",
                            obb=OB, n=N_LOC, i=Ho, j=Wo)
                        pad = 1 if li in (0, 2) else 0
                        for bpar in (0, 1):
                            for dpar in (0, 1):
                                srcv = Sv[:, ob, :, bpar::2, dpar::2]
                                na, ncc = srcv.shape[2], srcv.shape[3]
                                if pad == 1:
                                    ipv, rs = (0, 0) if bpar == 0 else (1, 1)
                                    jpv, cs = (0, 0) if dpar == 0 else (1, 1)
                                else:
                                    ipv, rs = (0, 0) if bpar == 0 else (1, 0)
                                    jpv, cs = (0, 0) if dpar == 0 else (1, 0)
                                if li == 0:
                                    dst = dstv[:, ob, :, ipv, jpv,
                                               rs:rs + na, cs:cs + ncc]
                                else:
                                    dst = dstv[:, :, ipv, jpv,
                                               rs:rs + na, cs:cs + ncc]
                                apply_tsp(dst, srcv)
                        blk = act[li].ap().shape[1] // OB
                        tgt = act[li].ap()[:, ob * blk:(ob + 1) * blk]
                        nc.vector.tensor_scalar(tgt, tgt, 0.0, 6.0,
                                                OP.max, OP.min)
                    else:
                        dst_t = act[li] if li != 5 else act[5]
                        apply_tsp(dst_t.ap()[:, ob * M:(ob + 1) * M],
                                  S.ap()[:, ob * M:(ob + 1) * M])
                        tgt = dst_t.ap()[:, ob * M:(ob + 1) * M]
                        nc.vector.tensor_scalar(tgt, tgt, 0.0, 6.0,
                                                OP.max, OP.min)

                # thresholds
                tcol = lambda l: thr_sb[li].ap()[:, l:l + 1]
                ntcol = lambda l: thr_sb[li].ap()[:, Llv + l:Llv + l + 1]
                t0col = thr_sb[li].ap()[:, 2 * Llv:2 * Llv + 1]
                dcol = thr_sb[li].ap()[0:1, 2 * Llv + 1:2 * Llv + 2]

                kk = L["k"] * L["k"]

                for pi, (poff, chunks) in enumerate(PIECES[li]):
                    n_arg = pi if npieces > 1 else None
                    Mp = sum(chunks)
                    Mp_pad = (Mp + 15) // 16 * 16
                    coff = [sum(chunks[:c]) for c in range(len(chunks))]
                    nchunk = len(chunks)

                    # ---- PSUM tiles: G slots reserved first, then conv ----
                    fp8 = FP8[li]
                    gps = [psp.tile([128, chunks[c]], F32, tag="ps",
                                    name=f"gps{li}_{pi}_{c}",
                                    padded_shape=[128, 512])
                           for c in range(nchunk)]
                    pss = {}
                    for ob in range(OB):
                        for c in range(nchunk):
                            pss[(ob, c)] = psp.tile(
                                [128, chunks[c]], F32, tag="ps",
                                name=f"ps{li}_{pi}_{ob}_{c}",
                                padded_shape=[128, 512])

                    GV = 8  # levels per V DMA (HWDGE fixed cost is per instr)
                    prod_ctr = 0
                    for kb in range(KB):
                        # previous layer's BN-apply for channel-block cb must
                        # be issued before any read of that act block
                        pop_applies(kb // kk)
                        win = src_window(L, kb, n_arg)
                        # G accumulation interleaved with production so the
                        # serial abs->add chain never head-blocks the queues
                        if kb == 0:
                            nc.scalar.activation(
                                gacc.ap()[:, :Mp], win, AF.Abs,
                                bias=t0col, scale=1.0)
                        else:
                            nc.scalar.activation(
                                gtmp.ap()[:, :Mp], win, AF.Abs,
                                bias=t0col, scale=1.0)
                            nc.vector.tensor_tensor(
                                gacc.ap()[:, :Mp], gacc.ap()[:, :Mp],
                                gtmp.ap()[:, :Mp], op=OP.add)
                        for g0 in range(0, Llv, GV):
                            gN = min(GV, Llv - g0)
                            vdt = F8 if fp8 else BF16
                            vt = vp.tile([128, gN * Cout], vdt, tag="v")
                            nc.sync.dma_start(
                                vt.rearrange("p (g c) -> p g c", g=gN),
                                w_d[li].ap()[kb * Llv + g0:
                                             kb * Llv + g0 + gN].rearrange(
                                    "g p c -> p g c"))
                            vv = vt.rearrange("p (g c) -> p g c", g=gN)
                            if fp8:
                                # level pairs -> DoubleRow matmuls (128 rows)
                                for i2 in range(0, gN, 2):
                                    scr = u8p.tile([128, 2 * Mp_pad], F8,
                                                   tag="u8")
                                    for t in (0, 1):
                                        l = g0 + i2 + t
                                        dst = scr[:, t * Mp_pad:
                                                  t * Mp_pad + Mp]
                                        p = PRODPAT[(kb * Llv + l) % 16]
                                        if p == "A":
                                            nc.scalar.activation(
                                                dst, win, AF.Sign,
                                                bias=ntcol(l), scale=1.0)
                                        else:
                                            eng = (nc.gpsimd if p == "P"
                                                   else nc.vector)
                                            eng.tensor_scalar(
                                                dst, win, tcol(l), None,
                                                op0=OP.is_gt)
                                    sv = scr.rearrange("p (t m) -> p t m",
                                                       t=2)
                                    first = (kb == 0 and g0 + i2 == 0)
                                    for ob in range(OB):
                                        lhsT = vv[:, i2:i2 + 2,
                                                  ob * 128:(ob + 1) * 128]
                                        for c in range(nchunk):
                                            c0, cN = coff[c], chunks[c]
                                            nc.tensor.matmul(
                                                pss[(ob, c)][:, :cN],
                                                lhsT,
                                                sv[:, :, c0:c0 + cN],
                                                start=first, stop=False,
                                                perf_mode=DR,
                                                skip_group_check=True)
                            else:
                                for l in range(g0, g0 + gN):
                                    vo = (l - g0) * Cout
                                    ut = up.tile([128, Mp_pad], BF16,
                                                 tag="u")
                                    nc.vector.tensor_scalar(
                                        ut[:, :Mp], win, tcol(l), None,
                                        op0=OP.is_gt)
                                    first = (kb == 0 and l == 0)
                                    for ob in range(OB):
                                        for c in range(nchunk):
                                            c0, cN = coff[c], chunks[c]
                                            nc.tensor.matmul(
                                                pss[(ob, c)][:, :cN],
                                                vt[:, vo + ob * 128:
                                                   vo + (ob + 1) * 128],
                                                ut[:, c0:c0 + cN],
                                                start=first, stop=False,
                                                skip_group_check=True)

                    # ---- G reduce to row + scale to g' (per chunk) ----
                    for c in range(nchunk):
                        c0, cN = coff[c], chunks[c]
                        nc.tensor.matmul(
                            gps[c][0:1, :cN], ones1f.ap(),
                            gacc.ap()[:, c0:c0 + cN],
                            start=True, stop=True, skip_group_check=True)
                        nc.vector.tensor_scalar_mul(
                            grow.ap()[0:1, poff + c0:poff + c0 + cN],
                            gps[c][0:1, :cN], dcol)

                    # ---- broadcast g' into all psum tiles, evacuate ----
                    for ob in range(OB):
                        for c in range(nchunk):
                            c0, cN = coff[c], chunks[c]
                            nc.tensor.matmul(
                                pss[(ob, c)][:, :cN], onesrf.ap()[0:1, :],
                                grow.ap()[0:1, poff + c0:poff + c0 + cN],
                                start=False, stop=True,
                                skip_group_check=True)
                            # evacuate on Act (Identity) to keep DVE free
                            nc.scalar.activation(
                                S.ap()[:, ob * M + poff + c0:
                                       ob * M + poff + c0 + cN],
                                pss[(ob, c)][:, :cN], AF.Identity)
                        if pi == npieces - 1:
                            finish_stats(ob)
                            pending.append((finish_apply, ob))

            for fa, ob_prev in pending:
                fa(ob_prev)
            pending.clear()

            # ---------------- final output DMA ----------------
            hw = LL["Ho"] * LL["Wo"]
            finv = act[5].ap().rearrange("p (ob n hw) -> p ob n hw",
                                         ob=LL["OB"], n=N_LOC, hw=hw)
            dst = out_d.ap().rearrange("n (ob p) h w -> p ob n (h w)",
                                       ob=LL["OB"], p=128)
            for ob in range(LL["OB"]):
                nc.sync.dma_start(dst[:, ob], finv[:, ob])

    nc.compile()
    return nc


def _get_nc():
    if "nc" not in _NC_CACHE:
        _NC_CACHE["nc"] = _build_nc()
    return _NC_CACHE["nc"]


def kernel(**inputs) -> np.ndarray:
    import time as _time
    from concourse.bass_utils import run_bass_kernel_spmd

    nc = _get_nc()
    in_maps = _host_prep(inputs)
    last = None
    for attempt in range(3):
        try:
            res = run_bass_kernel_spmd(nc, in_maps, core_ids=list(range(N_CORES)))
            outs = [np.asarray(r["out"]).reshape(N_LOC, 256, 4, 4)
                    for r in res.results]
            return np.concatenate(outs, axis=0).astype(np.float32)
        except Exception as e:  # transient axon mesh desync: wait + retry
            last = e
            _time.sleep(20 * (attempt + 1))
    raise last


if __name__ == "__main__":
    nc = _build_nc()
    print("build + compile OK")


# revision 46
# speedup vs baseline: 62.2260x; 1.0955x over previous
"""AdderNet CNN (6x adder_conv + sync-BN + ReLU6) on 8 Trainium2 NeuronCores.

v4: thermometer-quantization.  |x-w| = x + w - 2*min(x,w), and
min(x,w) - t0 = integral of 1[t<x]*1[t<w] over the weight range
~= sum_l Delta * u_l(x) * v_l(w) with u_l = 1[x > t_l], v_l = 1[w > t_l]
on a midpoint grid t_l = t0 + (l+0.5)*Delta covering only the (clipped)
weight range.  This is a DENSE matmul over binary features with
contraction dim 128*L per tap -- one lhsT [128, Cout] per (tap, level)
covers ALL output channels, replacing the per-channel one-hot matmuls:

  S_o(m) = P_o(m) + g'(m) (+ per-channel const, which BN cancels; BN is
  also per-channel scale-invariant so the 2*Delta factor drops out)
  P[o,m] = sum_{kb,l} V[kb,l,:,o] . u[kb,l,:,m]   (integer counts, exact)
  g'(m)  = -G(m)/(2*Delta),  G(m) = sum_k |x_k(m) - t0|  (f32 pathway)

Quantization error for taps with x > w depends only on w -> per-channel
constant -> cancelled by BN.  Data-dependent noise only from taps with
x inside the tiny clipped weight range; grid is 0-aligned so the
post-ReLU6 point mass at x=0 is exact.  Weights are clipped to
+-3 sigma (the clip residual is again a BN-cancelled constant).

Per layer: levels L = [48,16,16,16,16,16]; binary V tiles (bf16) are
streamed from DRAM; u tiles produced on DVE (4x mode, 0.26ns/col);
G accumulated in f32 (Act abs + DVE adds, one f32 ones-matmul reduce
per chunk, f32 ones-broadcast back into PSUM).  BN stats/apply and the
tiny per-ob (sum,sumsq) AllReduce are unchanged from v3.

Sharding: data-parallel over batch (2 images/core), sync-BN via
AllReduce of per-channel (sum, sumsq) per layer.
"""

import sys
import numpy as np

if "/opt/trn_rl_repo" not in sys.path:
    sys.path.insert(0, "/opt/trn_rl_repo")

import ml_dtypes

N_CORES = 8
N_LOC = 2  # images per core
EPS = 1e-5

# (Cin, Cout, k, stride, pad, Hi, Wi, Ho, Wo)
LAYER_SHAPES = [
    (512, 256, 1, 1, 0, 38, 38, 38, 38),
    (256, 512, 3, 2, 1, 38, 38, 19, 19),
    (512, 128, 1, 1, 0, 19, 19, 19, 19),
    (128, 256, 3, 2, 1, 19, 19, 10, 10),
    (256, 128, 1, 1, 0, 10, 10, 10, 10),
    (128, 256, 3, 2, 0, 10, 10, 4, 4),
]

# thermometer levels per layer (even) and weight-clip in sigmas
LEVELS = [36, 12, 12, 12, 12, 12]
WCLIP = 3.0
# layers using fp8 DoubleRow matmuls (PE at 2x; u tiles + V in fp8)
FP8 = [False, False, False, False, False, False]
# producer assignment per (kb*L + l) % 16 for fp8 layers.  D=DVE is_gt
# {0,1}, P=Pool is_gt {0,1}, A=Act Sign {-1,+1} with V scaled by 0.5 on
# the host (the -0.5*sum(V) constant is per-channel -> BN cancels it).
PRODPAT = "DADPDADDADPDADDP"

# per-layer piece/chunk structure: list of (piece_col_offset, [chunk sizes])
# pieces split M so that the live PSUM tiles fit the 8-bank budget
# (fp8-DR doubles the tile count: DR writes land in rows 0:64 only).
# L1 splits by image half, L2 by image n.
PIECES = [
    [(0, [481, 481]), (962, [481, 481]), (1924, [482, 482])],
    [(0, [361]), (361, [361])],
    [(0, [361, 361])],
    [(0, [200])],
    [(0, [200])],
    [(0, [32])],
]

SQW = 1456  # f32 stats scratch cols (stats computed in <=2 pieces)

_NC_CACHE = {}


def _grid(w, L):
    """Midpoint grid over the clipped weight range, 0-aligned.
    Returns (t0, Delta, thresholds[L])."""
    c = float(min(np.abs(w).max() * 1.0001, WCLIP * w.std()))
    j0 = L // 2
    Delta = c / j0
    t0 = -j0 * Delta
    t = t0 + (np.arange(L, dtype=np.float64) + 0.5) * Delta
    return t0, Delta, t.astype(np.float32)


def _host_prep(inputs, shapes=LAYER_SHAPES, n_total=16):
    """Build per-core in_maps from the raw reference inputs."""
    bf16 = ml_dtypes.bfloat16
    x = np.asarray(inputs["x"], dtype=np.float32)
    maps_common = {}
    for li, (Cin, Cout, k, *_rest) in enumerate(shapes, start=1):
        w = np.asarray(inputs[f"w{li}"], dtype=np.float32)  # [Cout,Cin,k,k]
        CB = (Cin + 127) // 128
        if k == 1:
            wt = w[:, :, 0, 0].T.reshape(CB, 128, Cout)
        else:
            # kb = cb*9 + dy*3 + dx
            wt = (
                w.transpose(1, 2, 3, 0)  # [Cin, k, k, Cout]
                .reshape(CB, 128, k * k, Cout)
                .transpose(0, 2, 1, 3)  # [CB, k*k, 128, Cout]
                .reshape(CB * k * k, 128, Cout)
            )
        L = LEVELS[li - 1]
        t0, Delta, t = _grid(wt, L)
        # V[kb*L + l, c, o] = 1[w > t_l], binary (fp8 for DR layers)
        KB = wt.shape[0]
        V = (wt[:, None, :, :] > t[None, :, None, None])  # [KB, L, 128, Cout]
        vdt = ml_dtypes.float8_e4m3 if FP8[li - 1] else bf16
        if k == 1:
            # [L, 128, KB*Cout]: one contiguous DMA per level covering all
            # taps (production is also merged into one instr per level)
            Vf = V.transpose(1, 2, 0, 3).reshape(L, 128, KB * Cout)
            Vf = Vf.astype(np.float32)
        else:
            Vf = V.reshape(KB * L, 128, Cout).astype(np.float32)
            if FP8[li - 1]:
                for kl in range(KB * L):
                    if PRODPAT[kl % 16] == "A":
                        Vf[kl] *= 0.5
        maps_common[f"w{li}q"] = np.ascontiguousarray(Vf.astype(vdt))
        thr = np.zeros((128, 2 * L + 2), np.float32)
        thr[:, :L] = t[None, :]
        thr[:, L:2 * L] = -t[None, :]
        thr[:, 2 * L] = -t0
        thr[:, 2 * L + 1] = -1.0 / (2.0 * Delta)
        maps_common[f"thr{li}"] = thr
        OB = (Cout + 127) // 128
        g = np.asarray(inputs[f"g{li}"], dtype=np.float32).reshape(OB, 128).T
        b = np.asarray(inputs[f"b{li}"], dtype=np.float32).reshape(OB, 128).T
        maps_common[f"g{li}"] = np.ascontiguousarray(g)
        maps_common[f"b{li}"] = np.ascontiguousarray(b)

    Cin0, _, _, _, _, Hi, Wi, _, _ = shapes[0]
    CB0 = (Cin0 + 127) // 128
    in_maps = []
    n_cores = n_total // N_LOC
    for c in range(n_cores):
        xs = x[c * N_LOC : (c + 1) * N_LOC]  # [2, Cin, H, W]
        xt = (
            xs.transpose(1, 0, 2, 3)
            .reshape(CB0, 128, N_LOC * Hi * Wi)
            .astype(bf16)
        )
        m = dict(maps_common)
        m["xt"] = np.ascontiguousarray(xt)
        in_maps.append(m)
    return in_maps


def _build_nc(shapes=LAYER_SHAPES, n_total=16, n_cores=N_CORES, mock_cc=False):
    import concourse.bacc as bacc
    import concourse.mybir as mybir
    from concourse import tile

    OP = mybir.AluOpType
    AF = mybir.ActivationFunctionType
    BF16 = mybir.dt.bfloat16
    F32 = mybir.dt.float32
    F8 = mybir.dt.float8e4
    DR = mybir.MatmulPerfMode.DoubleRow

    nc = bacc.Bacc(
        "TRN2",
        target_bir_lowering=False,
        debug=False,
        enable_asserts=False,
        num_devices=n_cores,
    )

    cfg = []
    for li, (Cin, Cout, k, stride, pad, Hi, Wi, Ho, Wo) in enumerate(shapes):
        CB = (Cin + 127) // 128
        OB = (Cout + 127) // 128
        KB = CB * k * k
        M = N_LOC * Ho * Wo
        Mpad = (M + 15) // 16 * 16
        cfg.append(
            dict(
                li=li, Cin=Cin, Cout=Cout, k=k, stride=stride, pad=pad,
                Hi=Hi, Wi=Wi, Ho=Ho, Wo=Wo, CB=CB, OB=OB, KB=KB, M=M,
                Mpad=Mpad, NM=n_total * Ho * Wo, L=LEVELS[li],
            )
        )

    # ---------------- DRAM I/O ----------------
    L1 = cfg[0]
    xt_d = nc.dram_tensor("xt", [L1["CB"], 128, N_LOC * L1["Hi"] * L1["Wi"]], BF16,
                          kind="ExternalInput")
    w_d, g_d, b_d, thr_d = [], [], [], []
    for L in cfg:
        li = L["li"] + 1
        vdt = F8 if FP8[L["li"]] else BF16
        if L["k"] == 1:
            wshape = [L["L"], 128, L["KB"] * L["Cout"]]
        else:
            wshape = [L["KB"] * L["L"], 128, L["Cout"]]
        w_d.append(nc.dram_tensor(f"w{li}q", wshape, vdt,
                                  kind="ExternalInput"))
        thr_d.append(nc.dram_tensor(f"thr{li}", [128, 2 * L["L"] + 2], F32,
                                    kind="ExternalInput"))
        g_d.append(nc.dram_tensor(f"g{li}", [128, L["OB"]], F32, kind="ExternalInput"))
        b_d.append(nc.dram_tensor(f"b{li}", [128, L["OB"]], F32, kind="ExternalInput"))
    LL = cfg[-1]
    out_d = nc.dram_tensor("out", [N_LOC, LL["Cout"], LL["Ho"], LL["Wo"]], F32,
                           kind="ExternalOutput")

    # ---------------- persistent SBUF ----------------
    def sb(name, cols, dt):
        return nc.alloc_sbuf_tensor(name, [128, cols], dt)

    xsb = sb("xsb", L1["CB"] * N_LOC * L1["Hi"] * L1["Wi"], BF16)

    act = []
    act.append(sb("act1p", cfg[0]["OB"] * N_LOC * 2 * 2 * 20 * 20, BF16))
    act.append(sb("act2", cfg[1]["OB"] * N_LOC * 19 * 19, BF16))
    act.append(sb("act3p", N_LOC * 2 * 2 * 11 * 12, BF16))
    act.append(sb("act4", cfg[3]["OB"] * N_LOC * 10 * 10, BF16))
    act.append(sb("act5p", N_LOC * 2 * 2 * 5 * 6, BF16))
    act.append(sb("fin6", cfg[5]["OB"] * N_LOC * 4 * 4, F32))

    S_max = max(L["OB"] * L["M"] for L in cfg)
    S = sb("S", S_max, F32)

    thr_sb = [sb(f"thrsb{L['li']}", 2 * L["L"] + 2, F32) for L in cfg]
    g_sb = [sb(f"gsb{L['li']}", L["OB"], F32) for L in cfg]
    b_sb = [sb(f"bsb{L['li']}", L["OB"], F32) for L in cfg]

    # f32 ones for the G pathway
    ones1f = sb("ones1f", 1, F32)       # [128,1] column (reduce lhsT)
    onesrf = sb("onesrf", 128, F32)     # row 0 used as [1,128] (bcast lhsT)
    # G accumulation + g' row
    Mmax = max(L["M"] for L in cfg)
    gacc = sb("gacc", Mmax, F32)
    gtmp = sb("gtmp", max(max(L["KB"] * L["M"] for L in cfg if L["k"] == 1), Mmax), F32)
    grow = sb("grow", Mmax, F32)        # g' = -G/(2*Delta), row 0

    stats_sb, stats_g, abwork = {}, {}, {}
    sqfull = sb("sqfull", SQW, F32)
    spart = sb("spart", 8, F32)

    with tile.TileContext(nc) as tc:
        # ---------------- input loads ----------------
        for cb in range(L1["CB"]):
            W0 = N_LOC * L1["Hi"] * L1["Wi"]
            nc.sync.dma_start(xsb.ap()[:, cb * W0:(cb + 1) * W0], xt_d.ap()[cb])
        for L in cfg:
            li = L["li"]
            nc.sync.dma_start(thr_sb[li].ap(), thr_d[li].ap())
            nc.sync.dma_start(g_sb[li].ap(), g_d[li].ap())
            nc.sync.dma_start(b_sb[li].ap(), b_d[li].ap())
        nc.vector.memset(ones1f.ap(), 1.0)
        nc.vector.memset(onesrf.ap(), 1.0)
        nc.gpsimd.memset(act[0].ap(), 0.0)
        nc.gpsimd.memset(act[2].ap(), 0.0)
        nc.gpsimd.memset(act[4].ap(), 0.0)

        with (
            tc.tile_pool(name="u", bufs=4) as up,
            tc.tile_pool(name="u8", bufs=6) as u8p,
            tc.tile_pool(name="v", bufs=4) as vp,
            tc.tile_pool(name="ps", bufs=8, space="PSUM") as psp,
            tc.tile_pool(name="dram", bufs=2, space="DRAM") as dramp,
        ):
            # ============ source window AP per (layer, kb[, piece]) ========
            def src_window(L, kb, n=None):
                """Window for tap kb; n=None -> full M, else image-piece n."""
                li = L["li"]
                k = L["k"]
                M = L["M"]
                if li == 0:
                    W0 = N_LOC * L["Hi"] * L["Wi"]
                    v = xsb.ap()[:, kb * W0:(kb + 1) * W0]
                    if n is None:
                        return v
                    poff, chks = PIECES[0][n]
                    return v[:, poff:poff + sum(chks)]
                if k == 1:
                    src = act[li - 1]
                    return src.ap()[:, kb * M:(kb + 1) * M]
                if li == 1:
                    src, CBv, R, C = act[0], L["CB"], 20, 20
                elif li == 3:
                    src, CBv, R, C = act[2], 1, 11, 12
                else:
                    src, CBv, R, C = act[4], 1, 5, 6
                Ho, Wo, pad = L["Ho"], L["Wo"], L["pad"]
                if CBv > 1:
                    v = src.ap().rearrange(
                        "p (cb n ip jp r c) -> p cb n ip jp r c",
                        cb=CBv, n=N_LOC, ip=2, jp=2, r=R, c=C)
                else:
                    v = src.ap().rearrange(
                        "p (n ip jp r c) -> p n ip jp r c",
                        n=N_LOC, ip=2, jp=2, r=R, c=C)

                def sel(d):
                    if pad == 1:
                        return (1, 0, 0) if d == 0 else ((0, 0, 0) if d == 1 else (1, 1, 1))
                    return (0, 0, 0) if d == 0 else ((1, 0, 0) if d == 1 else (0, 1, 1))

                cb, r9 = divmod(kb, 9)
                dy, dx = divmod(r9, 3)
                ipv, rs, _ = sel(dy)
                jpv, _, cs = sel(dx)
                if CBv > 1:
                    w = v[:, cb, :, ipv, jpv, rs:rs + Ho, cs:cs + Wo]
                else:
                    w = v[:, :, ipv, jpv, rs:rs + Ho, cs:cs + Wo]
                if n is None:
                    return w
                return w[:, n]

            # deferred BN-applies of the previous layer: (apply_fn, ob).
            # Popped at the next layer's G-phase just before the first read
            # of the corresponding act channel-block, hiding the AllReduce
            # round-trip behind the G/production work of earlier blocks.
            pending = []

            def pop_applies(upto_ob):
                while pending and pending[0][1] <= upto_ob:
                    fa, ob_prev = pending.pop(0)
                    fa(ob_prev)

            # ============ conv layers ============
            for L in cfg:
                li, Cout, KB, M, Mpad, OB, Llv = (
                    L["li"], L["Cout"], L["KB"], L["M"], L["Mpad"], L["OB"],
                    L["L"])
                stats_sb[li] = sb(f"stats{li}", 2 * OB, F32)
                stats_g[li] = sb(f"statsg{li}", 2 * OB, F32)
                abwork[li] = sb(f"abw{li}", 12 * OB, F32)

                Ho, Wo = L["Ho"], L["Wo"]
                npieces = len(PIECES[li])

                def finish_stats(ob, li=li, L=L, M=M, OB=OB):
                    # stats (in <=2 pieces) + AllReduce launch
                    npiece = (M + SQW - 1) // SQW
                    for h in range(npiece):
                        h0 = h * SQW
                        hN = min(SQW, M - h0)
                        src = S.ap()[:, ob * M + h0:ob * M + h0 + hN]
                        nc.vector.tensor_scalar(
                            sqfull.ap()[:, :hN], src, 0.0, 0.0,
                            op0=OP.add, op1=OP.add,
                            accum_out=spart.ap()[:, h:h + 1],
                        )
                        nc.scalar.activation(
                            sqfull.ap()[:, :hN], src, AF.Square,
                            accum_out=spart.ap()[:, 4 + h:5 + h],
                        )
                    if npiece == 1:
                        nc.vector.tensor_scalar_add(
                            stats_sb[li].ap()[:, 2 * ob:2 * ob + 1],
                            spart.ap()[:, 0:1], 0.0)
                        nc.vector.tensor_scalar_add(
                            stats_sb[li].ap()[:, 2 * ob + 1:2 * ob + 2],
                            spart.ap()[:, 4:5], 0.0)
                    else:
                        nc.vector.tensor_tensor(
                            stats_sb[li].ap()[:, 2 * ob:2 * ob + 1],
                            spart.ap()[:, 0:1], spart.ap()[:, 1:2],
                            op=OP.add)
                        nc.vector.tensor_tensor(
                            stats_sb[li].ap()[:, 2 * ob + 1:2 * ob + 2],
                            spart.ap()[:, 4:5], spart.ap()[:, 5:6],
                            op=OP.add)
                    if ob == OB - 1:
                        layer_allreduce()

                def layer_allreduce(li=li, OB=OB):
                    # one batched AllReduce per layer for all (sum, sumsq)
                    # columns; rides the gpsimd (Pool) queue so the wait
                    # never blocks the V-prefetch stream on the SP queue.
                    sti = dramp.tile([128, 2 * OB], F32, tag=f"sti{li}",
                                     name=f"sti{li}")
                    sto = dramp.tile([128, 2 * OB], F32, tag=f"sto{li}",
                                     name=f"sto{li}")
                    nc.gpsimd.dma_start(sti[:, :], stats_sb[li].ap())
                    if mock_cc:
                        nc.gpsimd.dma_start(sto[:, :], sti[:, :])
                    else:
                        nc.gpsimd.collective_compute(
                            "AllReduce", OP.add,
                            replica_groups=[list(range(n_cores))],
                            ins=[sti.opt()], outs=[sto.opt()],
                        )
                    nc.gpsimd.dma_start(stats_g[li].ap(), sto[:, :])

                def finish_apply(ob, li=li, L=L, M=M, OB=OB, Ho=Ho, Wo=Wo):
                    aw = abwork[li].ap()

                    def col(i):
                        return aw[:, i * OB + ob:i * OB + ob + 1]

                    (mean, ex2, m2, vpe, sq_, rc, u, s2, rinv, ga, _unused,
                     Bv) = [col(i) for i in range(12)]
                    inm = 1.0 / L["NM"]
                    nc.vector.tensor_scalar_mul(
                        mean, stats_g[li].ap()[:, 2 * ob:2 * ob + 1], inm)
                    nc.vector.tensor_scalar_mul(
                        ex2, stats_g[li].ap()[:, 2 * ob + 1:2 * ob + 2], inm)
                    nc.scalar.activation(m2, mean, AF.Square)
                    nc.vector.tensor_tensor(vpe, ex2, m2, op=OP.subtract)
                    nc.vector.tensor_scalar_add(vpe, vpe, EPS)
                    nc.scalar.activation(sq_, vpe, AF.Sqrt)
                    nc.vector.reciprocal(rc, sq_)
                    nc.vector.tensor_tensor(u, vpe, rc, op=OP.mult)
                    nc.vector.tensor_tensor(s2, sq_, u, op=OP.add)
                    nc.vector.tensor_scalar_mul(s2, s2, 0.5)
                    nc.vector.reciprocal(rinv, s2)
                    gcol = g_sb[li].ap()[:, ob:ob + 1]
                    bcol = b_sb[li].ap()[:, ob:ob + 1]
                    nc.vector.tensor_tensor(ga, gcol, rinv, op=OP.mult)
                    # y = A*S + (b - A*mean)
                    nc.vector.tensor_tensor(Bv, mean, ga, op=OP.mult)
                    nc.vector.tensor_tensor(Bv, bcol, Bv, op=OP.subtract)

                    def apply_tsp(dst_ap, src_ap):
                        nc.vector.tensor_scalar(
                            dst_ap, src_ap, ga, Bv, OP.mult, OP.add)

                    if li in (0, 2, 4):
                        if li == 0:
                            R, C = 20, 20
                            dstv = act[0].ap().rearrange(
                                "p (obb n ip jp r c) -> p obb n ip jp r c",
                                obb=OB, n=N_LOC, ip=2, jp=2, r=R, c=C)
                        elif li == 2:
                            R, C = 11, 12
                            dstv = act[2].ap().rearrange(
                                "p (n ip jp r c) -> p n ip jp r c",
                                n=N_LOC, ip=2, jp=2, r=R, c=C)
                        else:
                            R, C = 5, 6
                            dstv = act[4].ap().rearrange(
                                "p (n ip jp r c) -> p n ip jp r c",
                                n=N_LOC, ip=2, jp=2, r=R, c=C)
                        Sv = S.ap()[:, :OB * M].rearrange(
                            "p (obb n i j) -> p obb n i j",
                            obb=OB, n=N_LOC, i=Ho, j=Wo)
                        pad = 1 if li in (0, 2) else 0
                        for bpar in (0, 1):
                            for dpar in (0, 1):
                                srcv = Sv[:, ob, :, bpar::2, dpar::2]
                                na, ncc = srcv.shape[2], srcv.shape[3]
                                if pad == 1:
                                    ipv, rs = (0, 0) if bpar == 0 else (1, 1)
                                    jpv, cs = (0, 0) if dpar == 0 else (1, 1)
                                else:
                                    ipv, rs = (0, 0) if bpar == 0 else (1, 0)
                                    jpv, cs = (0, 0) if dpar == 0 else (1, 0)
                                if li == 0:
                                    dst = dstv[:, ob, :, ipv, jpv,
                                               rs:rs + na, cs:cs + ncc]
                                else:
                                    dst = dstv[:, :, ipv, jpv,
                                               rs:rs + na, cs:cs + ncc]
                                apply_tsp(dst, srcv)
                        blk = act[li].ap().shape[1] // OB
                        tgt = act[li].ap()[:, ob * blk:(ob + 1) * blk]
                        nc.vector.tensor_scalar(tgt, tgt, 0.0, 6.0,
                                                OP.max, OP.min)
                    else:
                        dst_t = act[li] if li != 5 else act[5]
                        apply_tsp(dst_t.ap()[:, ob * M:(ob + 1) * M],
                                  S.ap()[:, ob * M:(ob + 1) * M])
                        tgt = dst_t.ap()[:, ob * M:(ob + 1) * M]
                        nc.vector.tensor_scalar(tgt, tgt, 0.0, 6.0,
                                                OP.max, OP.min)

                # thresholds
                tcol = lambda l: thr_sb[li].ap()[:, l:l + 1]
                ntcol = lambda l: thr_sb[li].ap()[:, Llv + l:Llv + l + 1]
                t0col = thr_sb[li].ap()[:, 2 * Llv:2 * Llv + 1]
                dcol = thr_sb[li].ap()[0:1, 2 * Llv + 1:2 * Llv + 2]

                kk = L["k"] * L["k"]

                for pi, (poff, chunks) in enumerate(PIECES[li]):
                    n_arg = pi if npieces > 1 else None
                    Mp = sum(chunks)
                    Mp_pad = (Mp + 15) // 16 * 16
                    coff = [sum(chunks[:c]) for c in range(len(chunks))]
                    nchunk = len(chunks)

                    # ---- PSUM tiles: G slots reserved first, then conv ----
                    fp8 = FP8[li]
                    gps = [psp.tile([128, chunks[c]], F32, tag="ps",
                                    name=f"gps{li}_{pi}_{c}",
                                    padded_shape=[128, 512])
                           for c in range(nchunk)]
                    pss = {}
                    for ob in range(OB):
                        for c in range(nchunk):
                            pss[(ob, c)] = psp.tile(
                                [128, chunks[c]], F32, tag="ps",
                                name=f"ps{li}_{pi}_{ob}_{c}",
                                padded_shape=[128, 512])

                    GV = 8  # levels per V DMA (HWDGE fixed cost is per instr)
                    prod_ctr = 0
                    if L["k"] == 1:
                        # merged path: one production instr + one V DMA per
                        # level covering ALL taps ([128, KB, Mp] windows)
                        pop_applies(L["CB"])
                        if li == 0:
                            wall = xsb.ap().rearrange(
                                "p (kb m) -> p kb m",
                                kb=KB)[:, :, poff:poff + Mp]
                        else:
                            wall = act[li - 1].ap().rearrange(
                                "p (kb m) -> p kb m",
                                kb=KB)[:, :, poff:poff + Mp]
                        # G: one abs over all taps, then tree adds
                        gt3 = gtmp.ap()[:, :KB * Mp].rearrange(
                            "p (kb m) -> p kb m", kb=KB)
                        nc.scalar.activation(gt3, wall, AF.Abs,
                                             bias=t0col, scale=1.0)
                        if KB == 2:
                            nc.vector.tensor_tensor(
                                gacc.ap()[:, :Mp], gt3[:, 0], gt3[:, 1],
                                op=OP.add)
                        else:
                            nc.vector.tensor_tensor(
                                gacc.ap()[:, :Mp], gt3[:, 0], gt3[:, 1],
                                op=OP.add)
                            nc.vector.tensor_tensor(
                                gt3[:, 0], gt3[:, 2], gt3[:, 3], op=OP.add)
                            nc.vector.tensor_tensor(
                                gacc.ap()[:, :Mp], gacc.ap()[:, :Mp],
                                gt3[:, 0], op=OP.add)
                        for l in range(Llv):
                            vt = vp.tile([128, KB * Cout], BF16, tag="v")
                            nc.sync.dma_start(vt[:, :], w_d[li].ap()[l])
                            ut = up.tile([128, KB * Mp_pad], BF16, tag="u")
                            ut3 = ut.rearrange("p (kb m) -> p kb m", kb=KB)
                            nc.vector.tensor_scalar(
                                ut3[:, :, :Mp], wall, tcol(l), None,
                                op0=OP.is_gt)
                            for kb in range(KB):
                                for ob in range(OB):
                                    for c in range(nchunk):
                                        c0, cN = coff[c], chunks[c]
                                        nc.tensor.matmul(
                                            pss[(ob, c)][:, :cN],
                                            vt[:, kb * Cout + ob * 128:
                                               kb * Cout + (ob + 1) * 128],
                                            ut[:, kb * Mp_pad + c0:
                                               kb * Mp_pad + c0 + cN],
                                            start=(l == 0 and kb == 0),
                                            stop=False,
                                            skip_group_check=True)
                        KB_loop = 0  # skip the generic per-tap loop
                    else:
                        KB_loop = KB
                    for kb in range(KB_loop):
                        # previous layer's BN-apply for channel-block cb must
                        # be issued before any read of that act block
                        pop_applies(kb // kk)
                        win = src_window(L, kb, n_arg)
                        # G accumulation interleaved with production so the
                        # serial abs->add chain never head-blocks the queues
                        if kb == 0:
                            nc.scalar.activation(
                                gacc.ap()[:, :Mp], win, AF.Abs,
                                bias=t0col, scale=1.0)
                        else:
                            nc.scalar.activation(
                                gtmp.ap()[:, :Mp], win, AF.Abs,
                                bias=t0col, scale=1.0)
                            nc.vector.tensor_tensor(
                                gacc.ap()[:, :Mp], gacc.ap()[:, :Mp],
                                gtmp.ap()[:, :Mp], op=OP.add)
                        for g0 in range(0, Llv, GV):
                            gN = min(GV, Llv - g0)
                            vdt = F8 if fp8 else BF16
                            vt = vp.tile([128, gN * Cout], vdt, tag="v")
                            nc.sync.dma_start(
                                vt.rearrange("p (g c) -> p g c", g=gN),
                                w_d[li].ap()[kb * Llv + g0:
                                             kb * Llv + g0 + gN].rearrange(
                                    "g p c -> p g c"))
                            vv = vt.rearrange("p (g c) -> p g c", g=gN)
                            if fp8:
                                # level pairs -> DoubleRow matmuls (128 rows)
                                for i2 in range(0, gN, 2):
                                    scr = u8p.tile([128, 2 * Mp_pad], F8,
                                                   tag="u8")
                                    for t in (0, 1):
                                        l = g0 + i2 + t
                                        dst = scr[:, t * Mp_pad:
                                                  t * Mp_pad + Mp]
                                        p = PRODPAT[(kb * Llv + l) % 16]
                                        if p == "A":
                                            nc.scalar.activation(
                                                dst, win, AF.Sign,
                                                bias=ntcol(l), scale=1.0)
                                        else:
                                            eng = (nc.gpsimd if p == "P"
                                                   else nc.vector)
                                            eng.tensor_scalar(
                                                dst, win, tcol(l), None,
                                                op0=OP.is_gt)
                                    sv = scr.rearrange("p (t m) -> p t m",
                                                       t=2)
                                    first = (kb == 0 and g0 + i2 == 0)
                                    for ob in range(OB):
                                        lhsT = vv[:, i2:i2 + 2,
                                                  ob * 128:(ob + 1) * 128]
                                        for c in range(nchunk):
                                            c0, cN = coff[c], chunks[c]
                                            nc.tensor.matmul(
                                                pss[(ob, c)][:, :cN],
                                                lhsT,
                                                sv[:, :, c0:c0 + cN],
                                                start=first, stop=False,
                                                perf_mode=DR,
                                                skip_group_check=True)
                            else:
                                for l in range(g0, g0 + gN):
                                    vo = (l - g0) * Cout
                                    ut = up.tile([128, Mp_pad], BF16,
                                                 tag="u")
                                    nc.vector.tensor_scalar(
                                        ut[:, :Mp], win, tcol(l), None,
                                        op0=OP.is_gt)
                                    first = (kb == 0 and l == 0)
                                    for ob in range(OB):
                                        for c in range(nchunk):
                                            c0, cN = coff[c], chunks[c]
                                            nc.tensor.matmul(
                                                pss[(ob, c)][:, :cN],
                                                vt[:, vo + ob * 128:
                                                   vo + (ob + 1) * 128],
                                                ut[:, c0:c0 + cN],
                                                start=first, stop=False,
                                                skip_group_check=True)

                    # ---- G reduce to row + scale to g' (per chunk) ----
                    for c in range(nchunk):
                        c0, cN = coff[c], chunks[c]
                        nc.tensor.matmul(
                            gps[c][0:1, :cN], ones1f.ap(),
                            gacc.ap()[:, c0:c0 + cN],
                            start=True, stop=True, skip_group_check=True)
                        nc.vector.tensor_scalar_mul(
                            grow.ap()[0:1, poff + c0:poff + c0 + cN],
                            gps[c][0:1, :cN], dcol)

                    # ---- broadcast g' into all psum tiles, evacuate ----
                    for ob in range(OB):
                        for c in range(nchunk):
                            c0, cN = coff[c], chunks[c]
                            nc.tensor.matmul(
                                pss[(ob, c)][:, :cN], onesrf.ap()[0:1, :],
                                grow.ap()[0:1, poff + c0:poff + c0 + cN],
                                start=False, stop=True,
                                skip_group_check=True)
                            # evacuate on Act (Identity) to keep DVE free
                            nc.scalar.activation(
                                S.ap()[:, ob * M + poff + c0:
                                       ob * M + poff + c0 + cN],
                                pss[(ob, c)][:, :cN], AF.Identity)
                        if pi == npieces - 1:
                            finish_stats(ob)
                            pending.append((finish_apply, ob))

            for fa, ob_prev in pending:
                fa(ob_prev)
            pending.clear()

            # ---------------- final output DMA ----------------
            hw = LL["Ho"] * LL["Wo"]
            finv = act[5].ap().rearrange("p (ob n hw) -> p ob n hw",
                                         ob=LL["OB"], n=N_LOC, hw=hw)
            dst = out_d.ap().rearrange("n (ob p) h w -> p ob n (h w)",
                                       ob=LL["OB"], p=128)
            for ob in range(LL["OB"]):
                nc.sync.dma_start(dst[:, ob], finv[:, ob])

    nc.compile()
    return nc


def _get_nc():
    if "nc" not in _NC_CACHE:
        _NC_CACHE["nc"] = _build_nc()
    return _NC_CACHE["nc"]


def kernel(**inputs) -> np.ndarray:
    import time as _time
    from concourse.bass_utils import run_bass_kernel_spmd

    nc = _get_nc()
    in_maps = _host_prep(inputs)
    last = None
    for attempt in range(3):
        try:
            res = run_bass_kernel_spmd(nc, in_maps, core_ids=list(range(N_CORES)))
            outs = [np.asarray(r["out"]).reshape(N_LOC, 256, 4, 4)
                    for r in res.results]
            return np.concatenate(outs, axis=0).astype(np.float32)
        except Exception as e:  # transient axon mesh desync: wait + retry
            last = e
            _time.sleep(20 * (attempt + 1))
    raise last


if __name__ == "__main__":
    nc = _build_nc()
    print("build + compile OK")
